# revision 1
# baseline (speedup 1.0000x reference)
"""Trainium2 Bass kernel for a dense transformer decoder block.

Problem: B=2, S=2048, H=2048, NH=16 (head_dim=128), FFN=8192, fp32.

Sharding (zero collectives): 8 cores = 2 batches x 4 query-chunks of 512
contiguous rows.  Every core redundantly computes LN1 + K/V projections for
its full batch (causality means late query chunks need all keys), then runs
attention for its own 512 queries against all 2048 keys (additive -1e4 mask
input reproduces the reference's causal mask exactly), followed by WO, LN2
and the FFN on its own rows.  The final output is disjoint across cores, so
the host just concatenates shards - no cross-core communication anywhere.

On-device layout is feature-major ([feature, seq] - the transpose of the
reference layout).  With weights pre-transposed on the host, every matmul in
the chain (QKV, scores^T, AV, WO, FFN1, FFN2) contracts over the partition
dimension with no on-device transposes; V is produced directly in
sequence-major layout by using the activations as the stationary operand.
LayerNorm/softmax statistics over the partition dim use ones-vector matmuls
on the PE.  All matmuls run as float32r (full-rate fp32, ~1e-4 relative
error); every matmul operand is written by its producer through a
float32r-bitcast AP, which the BIR verifier requires.
"""

import json

import numpy as np

import concourse.bass as bass
import concourse.bass2jax as bass2jax
import concourse.mybir as mybir
import concourse.tile as tile
from concourse.bass_utils import compile_bir_kernel as _orig_compile_bir_kernel
from concourse.bass_utils import run_bass_kernel_spmd

F32 = mybir.dt.float32
F32R = mybir.dt.float32r
AF = mybir.ActivationFunctionType
OP = mybir.AluOpType

B, S, H, NH, HD, FF = 2, 2048, 2048, 16, 128, 8192
P = 128
QR = 512            # query rows per core
HT = H // P         # 16 feature tiles
FT = FF // P        # 64 ffn tiles
EPS = 1e-5
NEG = -1e4

# ---------------------------------------------------------------------------
# Workaround for this container's walrus build: it supports only ONE sync
# wait per instruction, but Tile attaches several.  Rewrite the BIR just
# before walrus: an instruction with N>1 waits gets N-1 same-engine NoOps
# inserted before it, each carrying one wait (same-engine program order makes
# this equivalent).
# ---------------------------------------------------------------------------


def _split_multiwaits(bir_bytes):
    bir = json.loads(bir_bytes)
    ctr = 0
    for fn in bir.get("functions", []):
        for blk in fn.get("blocks", []):
            new = []
            for inst in blk.get("instructions", []):
                si = inst.get("sync_info")
                waits = (si or {}).get("on_wait") or []
                if len(waits) > 1:
                    for w in waits[:-1]:
                        ctr += 1
                        new.append({
                            "engine": inst["engine"],
                            "ins": [],
                            "outs": [],
                            "name": f"I-mwsplit{ctr}",
                            "opcode": "NoOp",
                            "sync_info": {"on_update": [], "on_wait": [w]},
                            "text_hint": "multiwait_split",
                        })
                    si["on_wait"] = [waits[-1]]
                new.append(inst)
            blk["instructions"] = new
    return json.dumps(bir).encode()


def _patched_compile_bir_kernel(bir_json, tmpdir, neff_name="file.neff", **kw):
    if isinstance(bir_json, str):
        bir_json = bir_json.encode()
    return _orig_compile_bir_kernel(_split_multiwaits(bir_json), tmpdir,
                                    neff_name=neff_name, **kw)


def _install_patch():
    bass2jax.compile_bir_kernel = _patched_compile_bir_kernel


def r(ap):
    """View an fp32 AP as float32r (full-rate PE mode)."""
    return ap.bitcast(F32R)


# ---------------------------------------------------------------------------
# Device program
# ---------------------------------------------------------------------------


def _ln_stats(nc, tc, pool, ones, ones_row, load_rhs, n, tag):
    """Partition-dim (feature-dim) layernorm stats via ones-matmuls.

    load_rhs: callable i -> AP [128, n], the i'th feature tile, whose
    producer already wrote it through a float32r AP.
    Returns (bmean, brstd) [128, n] tiles broadcast along partitions
    (broadcast = K=1 matmul with a [1,128] ones row as lhsT).
    """
    nch = n // 512
    mean = pool.tile([1, n], F32, tag=f"{tag}_mean", bufs=1)
    msq = pool.tile([1, n], F32, tag=f"{tag}_msq", bufs=1)
    m2 = pool.tile([1, n], F32, tag=f"{tag}_m2", bufs=1)
    rstd = pool.tile([1, n], F32, tag=f"{tag}_rstd", bufs=1)
    with tc.tile_pool(name=f"{tag}_sps", bufs=1, space="PSUM") as psum:
        mean_ps = [psum.tile([1, 512], F32, tag=f"{tag}_mps{c}",
                             name=f"{tag}_mps{c}") for c in range(nch)]
        sq_ps = [psum.tile([1, 512], F32, tag=f"{tag}_sps{c}",
                           name=f"{tag}_sps{c}") for c in range(nch)]
        for i in range(HT):
            xt = load_rhs(i)
            xsq = pool.tile([P, n], F32, tag=f"{tag}_sq", bufs=1)
            nc.scalar.activation(r(xsq[:]), xt, AF.Square)
            for c in range(nch):
                sl = slice(512 * c, 512 * (c + 1))
                nc.tensor.matmul(mean_ps[c][:], r(ones[:]), r(xt[:, sl]),
                                 start=(i == 0), stop=(i == HT - 1))
                nc.tensor.matmul(sq_ps[c][:], r(ones[:]), r(xsq[:, sl]),
                                 start=(i == 0), stop=(i == HT - 1))
        for c in range(nch):
            sl = slice(512 * c, 512 * (c + 1))
            nc.scalar.activation(r(mean[:, sl]), mean_ps[c][:], AF.Copy,
                                 scale=1.0 / H)
            nc.scalar.activation(msq[:, sl], sq_ps[c][:], AF.Copy,
                                 scale=1.0 / H)
    nc.vector.tensor_mul(m2[:], mean[:], mean[:])
    nc.vector.tensor_sub(m2[:], msq[:], m2[:])          # var
    nc.vector.tensor_scalar_add(m2[:], m2[:], EPS)
    nc.vector.reciprocal(m2[:], m2[:])                  # 1/(var+eps)
    nc.scalar.activation(r(rstd[:]), m2[:], AF.Sqrt)    # rsqrt(var+eps)
    bmean = pool.tile([P, n], F32, tag=f"{tag}_bmean", bufs=1)
    brstd = pool.tile([P, n], F32, tag=f"{tag}_brstd", bufs=1)
    with tc.tile_pool(name=f"{tag}_bps", bufs=1, space="PSUM") as bps:
        for c in range(nch):
            sl = slice(512 * c, 512 * (c + 1))
            mps = bps.tile([P, 512], F32, tag=f"{tag}_bmps{c}",
                           name=f"{tag}_bmps{c}")
            nc.tensor.matmul(mps[:], r(ones_row[:]), r(mean[:, sl]),
                             start=True, stop=True)
            nc.scalar.activation(bmean[:, sl], mps[:], AF.Copy)
            rps = bps.tile([P, 512], F32, tag=f"{tag}_brps{c}",
                           name=f"{tag}_brps{c}")
            nc.tensor.matmul(rps[:], r(ones_row[:]), r(rstd[:, sl]),
                             start=True, stop=True)
            nc.scalar.activation(brstd[:, sl], rps[:], AF.Copy)
    return bmean, brstd


def build_nc(debug_outputs=()):
    _install_patch()
    nc = bass.Bass("TRN2")

    xT = nc.dram_tensor("xT", (H, S), F32, kind="ExternalInput")
    xTq = nc.dram_tensor("xTq", (H, QR), F32, kind="ExternalInput")
    maskT = nc.dram_tensor("maskT", (S, QR), F32, kind="ExternalInput")
    ones_d = nc.dram_tensor("ones_d", (P, 1), F32, kind="ExternalInput")
    ones_r_d = nc.dram_tensor("ones_r_d", (1, P), F32, kind="ExternalInput")
    wq_t = nc.dram_tensor("wq_t", (HT, P, HT, P), F32, kind="ExternalInput")
    wk_t = nc.dram_tensor("wk_t", (HT, P, HT, P), F32, kind="ExternalInput")
    wvT = nc.dram_tensor("wvT", (H, H), F32, kind="ExternalInput")
    wo_t = nc.dram_tensor("wo_t", (HT, P, HT, P), F32, kind="ExternalInput")
    w1_t = nc.dram_tensor("w1_t", (FT, P, HT, P), F32, kind="ExternalInput")
    w2_t = nc.dram_tensor("w2_t", (HT, P, FT, P), F32, kind="ExternalInput")
    bq = nc.dram_tensor("bq", (H,), F32, kind="ExternalInput")
    bk = nc.dram_tensor("bk", (H,), F32, kind="ExternalInput")
    bv = nc.dram_tensor("bv", (H,), F32, kind="ExternalInput")
    bwo = nc.dram_tensor("bwo", (H,), F32, kind="ExternalInput")
    b1 = nc.dram_tensor("b1", (FF,), F32, kind="ExternalInput")
    b2 = nc.dram_tensor("b2", (H,), F32, kind="ExternalInput")
    ln1w = nc.dram_tensor("ln1w", (H,), F32, kind="ExternalInput")
    ln1b = nc.dram_tensor("ln1b", (H,), F32, kind="ExternalInput")
    ln2w = nc.dram_tensor("ln2w", (H,), F32, kind="ExternalInput")
    ln2b = nc.dram_tensor("ln2b", (H,), F32, kind="ExternalInput")
    outT = nc.dram_tensor("outT", (H, QR), F32, kind="ExternalOutput")

    dbg = {}
    for name, shape in dict(a=(H, S), k=(H, S), vT=(S, H), q=(H, QR),
                            av=(H, QR), h=(H, QR), g=(H, QR)).items():
        if name in debug_outputs:
            dbg[name] = nc.dram_tensor(f"dbg_{name}", shape, F32,
                                       kind="ExternalOutput")

    def dbg_dump(name, src3d):
        if name in dbg:
            for i in range(HT):
                nc.sync.dma_start(dbg[name][P * i:P * (i + 1), :],
                                  src3d[:, i, :])

    with tile.TileContext(nc) as tc:
        cm_const = tc.tile_pool(name="const", bufs=1)
        const = cm_const.__enter__()
        ones = const.tile([P, 1], F32, tag="ones")
        nc.sync.dma_start(r(ones[:]), r(ones_d[:]))
        ones_row = const.tile([1, P], F32, tag="ones_row")
        nc.sync.dma_start(r(ones_row[:]), r(ones_r_d[:]))

        def bias_tile(name, dram_t, ntiles):
            t = const.tile([P, ntiles], F32, tag=f"b_{name}")
            nc.sync.dma_start(t[:], dram_t.rearrange("(t p) -> p t", p=P))
            return t

        bq_t = bias_tile("bq", bq, HT)
        bk_t = bias_tile("bk", bk, HT)
        bv_t = bias_tile("bv", bv, HT)
        bwo_t = bias_tile("bwo", bwo, HT)
        b1_t = bias_tile("b1", b1, FT)
        b2_t = bias_tile("b2", b2, HT)
        ln1w_t = bias_tile("ln1w", ln1w, HT)
        ln1b_t = bias_tile("ln1b", ln1b, HT)
        ln2w_t = bias_tile("ln2w", ln2w, HT)
        ln2b_t = bias_tile("ln2b", ln2b, HT)

        cm_dram = tc.tile_pool(name="dram", bufs=1, space="DRAM")
        dram = cm_dram.__enter__()
        a_d = dram.tile([H, S], F32, tag="a")
        k_d = dram.tile([H, S], F32, tag="k")
        vT_d = dram.tile([S, H], F32, tag="vT")
        h_d = dram.tile([H, QR], F32, tag="h")

        # ============ S1+S2: LN1 over the full batch, fused in place =======
        cm_ares = tc.tile_pool(name="ares", bufs=1)
        arp = cm_ares.__enter__()
        a_res = arp.tile([P, HT, S], F32, tag="a_res")
        with tc.tile_pool(name="ln1", bufs=2) as lp:
            for i in range(HT):
                nc.sync.dma_start(r(a_res[:, i, :]), r(xT[P * i:P * (i + 1), :]))
            bmean, brstd = _ln_stats(nc, tc, lp, ones, ones_row,
                                     lambda i: a_res[:, i, :], S, "ln1")
            for i in range(HT):
                t1 = lp.tile([P, S], F32, tag="t1", bufs=1)
                nc.vector.tensor_sub(t1[:], a_res[:, i, :], bmean[:])
                nc.vector.tensor_mul(t1[:], t1[:], brstd[:])
                nc.vector.tensor_scalar(r(a_res[:, i, :]), t1[:],
                                        ln1w_t[:, i:i + 1], ln1b_t[:, i:i + 1],
                                        op0=OP.mult, op1=OP.add)
                nc.sync.dma_start(a_d[P * i:P * (i + 1), :], a_res[:, i, :])
                if "a" in dbg:
                    nc.sync.dma_start(dbg["a"][P * i:P * (i + 1), :],
                                      a_res[:, i, :])

        # ============ S4: K projection (a resident, WkT streamed) ===========
        with tc.tile_pool(name="kproj", bufs=2) as kp, \
             tc.tile_pool(name="kps", bufs=1, space="PSUM") as kps:
            for dM in range(HT):
                kw = kp.tile([P, HT, P], F32, tag="kw")
                nc.sync.dma_start(r(kw[:]), r(wk_t[dM]))
                ps = [kps.tile([P, 512], F32, tag=f"kp{n}", name=f"kp{n}")
                      for n in range(4)]
                for ht in range(HT):
                    for n in range(4):
                        nc.tensor.matmul(
                            ps[n][:], r(kw[:, ht, :]),
                            r(a_res[:, ht, 512 * n:512 * (n + 1)]),
                            start=(ht == 0), stop=(ht == HT - 1))
                kst = kp.tile([P, S], F32, tag="kst")
                for n in range(4):
                    nc.scalar.activation(kst[:, 512 * n:512 * (n + 1)],
                                         ps[n][:], AF.Identity,
                                         bias=bk_t[:, dM:dM + 1])
                nc.sync.dma_start(k_d[P * dM:P * (dM + 1), :], kst[:])
                if "k" in dbg:
                    nc.sync.dma_start(dbg["k"][P * dM:P * (dM + 1), :], kst[:])
        cm_ares.__exit__(None, None, None)

        # ============ S3: V^T projection (WvT resident, a streamed) =========
        with tc.tile_pool(name="vproj", bufs=2) as vp, \
             tc.tile_pool(name="wvres", bufs=1) as wvp, \
             tc.tile_pool(name="vps", bufs=1, space="PSUM") as vps:
            wv_res = wvp.tile([P, HT, H], F32, tag="wv_res")
            nc.sync.dma_start(r(wv_res[:]),
                              r(wvT.rearrange("(t p) d -> p t d", p=P)))
            for sM in range(HT):
                alh = vp.tile([P, HT, P], F32, tag="alh")
                nc.sync.dma_start(
                    r(alh[:]), r(a_d.rearrange("(t p) s -> p t s", p=P)
                                 [:, :, P * sM:P * (sM + 1)]))
                ps = [vps.tile([P, 512], F32, tag=f"vp{n}", name=f"vp{n}")
                      for n in range(4)]
                for ht in range(HT):
                    for n in range(4):
                        nc.tensor.matmul(
                            ps[n][:], r(alh[:, ht, :]),
                            r(wv_res[:, ht, 512 * n:512 * (n + 1)]),
                            start=(ht == 0), stop=(ht == HT - 1))
                vst = vp.tile([P, H], F32, tag="vst")
                for n in range(4):
                    nc.scalar.activation(vst[:, 512 * n:512 * (n + 1)],
                                         ps[n][:], AF.Copy)
                nc.sync.dma_start(vT_d[P * sM:P * (sM + 1), :], vst[:])
                if "vT" in dbg:
                    nc.sync.dma_start(dbg["vT"][P * sM:P * (sM + 1), :],
                                      vst[:])

        # ============ S4.5 + S5: LN1 on the q rows, then Q projection =======
        cm_qres = tc.tile_pool(name="qres", bufs=1)
        qres_p = cm_qres.__enter__()
        q_res = qres_p.tile([P, HT, QR], F32, tag="q_res")
        with tc.tile_pool(name="lnq", bufs=2) as lqp:
            xTq_sc = lqp.tile([P, HT, QR], F32, tag="xTq_sc", bufs=1)
            nc.sync.dma_start(r(xTq_sc[:]),
                              r(xTq.rearrange("(t p) s -> p t s", p=P)))
            bmean_q, brstd_q = _ln_stats(
                nc, tc, lqp, ones, ones_row,
                lambda i: xTq_sc[:, i, :], QR, "lnq")
            aq = lqp.tile([P, HT, QR], F32, tag="aq", bufs=1)
            for i in range(HT):
                t1 = lqp.tile([P, QR], F32, tag="t1")
                nc.vector.tensor_sub(t1[:], xTq_sc[:, i, :], bmean_q[:])
                nc.vector.tensor_mul(t1[:], t1[:], brstd_q[:])
                nc.vector.tensor_scalar(r(aq[:, i, :]), t1[:],
                                        ln1w_t[:, i:i + 1], ln1b_t[:, i:i + 1],
                                        op0=OP.mult, op1=OP.add)
            with tc.tile_pool(name="qproj", bufs=2) as qp, \
                 tc.tile_pool(name="qps", bufs=2, space="PSUM") as qps:
                for dM in range(HT):
                    qw = qp.tile([P, HT, P], F32, tag="qw")
                    nc.sync.dma_start(r(qw[:]), r(wq_t[dM]))
                    ps = qps.tile([P, QR], F32, tag="qpsum")
                    for ht in range(HT):
                        nc.tensor.matmul(ps[:], r(qw[:, ht, :]),
                                         r(aq[:, ht, :]),
                                         start=(ht == 0), stop=(ht == HT - 1))
                    nc.scalar.activation(r(q_res[:, dM, :]), ps[:],
                                         AF.Identity,
                                         bias=bq_t[:, dM:dM + 1])
                dbg_dump("q", q_res)

        # ============ S6: attention ========================================
        cm_av = tc.tile_pool(name="avres", bufs=1)
        av_p = cm_av.__enter__()
        av_res = av_p.tile([P, HT, QR], F32, tag="av_res")
        with tc.tile_pool(name="attn", bufs=2) as ap_, \
             tc.tile_pool(name="attn1", bufs=1) as ap1, \
             tc.tile_pool(name="attnps", bufs=2, space="PSUM") as aps:
            mask_res = ap1.tile([P, HT, QR], F32, tag="mask_res")
            nc.sync.dma_start(mask_res[:],
                              maskT.rearrange("(t p) s -> p t s", p=P))
            for hd_i in range(NH):
                kh = ap_.tile([P, S], F32, tag="kh")
                nc.sync.dma_start(r(kh[:]), r(k_d[P * hd_i:P * (hd_i + 1), :]))
                vh = ap_.tile([P, HT, P], F32, tag="vh")
                nc.sync.dma_start(
                    r(vh[:]), r(vT_d.rearrange("(t p) d -> p t d", p=P)
                                [:, :, P * hd_i:P * (hd_i + 1)]))
                pt = ap1.tile([P, HT, QR], F32, tag="pt")
                for kb in range(HT):
                    sp = aps.tile([P, QR], F32, tag="sp")
                    nc.tensor.matmul(sp[:], r(kh[:, P * kb:P * (kb + 1)]),
                                     r(q_res[:, hd_i, :]),
                                     start=True, stop=True)
                    ptmp = ap_.tile([P, QR], F32, tag="ptmp")
                    nc.vector.tensor_add(ptmp[:], sp[:], mask_res[:, kb, :])
                    nc.scalar.activation(r(pt[:, kb, :]), ptmp[:], AF.Exp)
                dn = aps.tile([1, QR], F32, tag="dn")
                for kb in range(HT):
                    nc.tensor.matmul(dn[:], r(ones[:]), r(pt[:, kb, :]),
                                     start=(kb == 0), stop=(kb == HT - 1))
                rec = ap_.tile([1, QR], F32, tag="rec")
                with nc.allow_low_precision(reason="f32r is fp32 bits"):
                    nc.vector.reciprocal(r(rec[:]), dn[:])
                brec_ps = aps.tile([P, QR], F32, tag="brec_ps")
                nc.tensor.matmul(brec_ps[:], r(ones_row[:]), r(rec[:]),
                                 start=True, stop=True)
                brec = ap_.tile([P, QR], F32, tag="brec")
                nc.scalar.activation(brec[:], brec_ps[:], AF.Copy)
                avp = aps.tile([P, QR], F32, tag="avp")
                for kb in range(HT):
                    nc.tensor.matmul(avp[:], r(vh[:, kb, :]), r(pt[:, kb, :]),
                                     start=(kb == 0), stop=(kb == HT - 1))
                nc.vector.tensor_mul(r(av_res[:, hd_i, :]), avp[:], brec[:])
                nc.vector.tensor_scalar_add(r(av_res[:, hd_i, :]),
                                            av_res[:, hd_i, :],
                                            bv_t[:, hd_i:hd_i + 1])
            dbg_dump("av", av_res)

        # ============ S7: WO + residual ====================================
        with tc.tile_pool(name="wo", bufs=2) as wop, \
             tc.tile_pool(name="wops", bufs=2, space="PSUM") as wops:
            for dM in range(HT):
                wot = wop.tile([P, HT, P], F32, tag="wot")
                nc.sync.dma_start(r(wot[:]), r(wo_t[dM]))
                xq_t = wop.tile([P, QR], F32, tag="xq_t")
                nc.sync.dma_start(xq_t[:], xTq[P * dM:P * (dM + 1), :])
                ps = wops.tile([P, QR], F32, tag="wopsum")
                for ht in range(HT):
                    nc.tensor.matmul(ps[:], r(wot[:, ht, :]),
                                     r(av_res[:, ht, :]),
                                     start=(ht == 0), stop=(ht == HT - 1))
                hst = wop.tile([P, QR], F32, tag="hst")
                nc.vector.scalar_tensor_tensor(
                    hst[:], ps[:], bwo_t[:, dM:dM + 1],
                    xq_t[:], op0=OP.add, op1=OP.add)
                nc.sync.dma_start(h_d[P * dM:P * (dM + 1), :], hst[:])
                if "h" in dbg:
                    nc.sync.dma_start(dbg["h"][P * dM:P * (dM + 1), :],
                                      hst[:])
        cm_av.__exit__(None, None, None)
        cm_qres.__exit__(None, None, None)

        # ============ S8: LN2 ==============================================
        cm_f = tc.tile_pool(name="fres", bufs=1)
        f_p = cm_f.__enter__()
        f_res = f_p.tile([P, FT, QR], F32, tag="f_res")
        cm_g = tc.tile_pool(name="gres", bufs=1)
        g_p = cm_g.__enter__()
        g_res = g_p.tile([P, HT, QR], F32, tag="g_res")
        with tc.tile_pool(name="ln2", bufs=2) as l2p:
            def ln2_load(i):
                ht_ = l2p.tile([P, QR], F32, tag="hl")
                nc.sync.dma_start(r(ht_[:]), r(h_d[P * i:P * (i + 1), :]))
                return ht_[:]

            bmean2, brstd2 = _ln_stats(
                nc, tc, l2p, ones, ones_row, ln2_load, QR, "ln2")
            for i in range(HT):
                hl2 = ln2_load(i)
                t1 = l2p.tile([P, QR], F32, tag="t1")
                nc.vector.tensor_sub(t1[:], hl2, bmean2[:])
                nc.vector.tensor_mul(t1[:], t1[:], brstd2[:])
                nc.vector.tensor_scalar(r(g_res[:, i, :]), t1[:],
                                        ln2w_t[:, i:i + 1], ln2b_t[:, i:i + 1],
                                        op0=OP.mult, op1=OP.add)
            dbg_dump("g", g_res)

        # ============ S9: FFN1 + gelu ======================================
        with tc.tile_pool(name="ffn1", bufs=2) as f1p, \
             tc.tile_pool(name="f1ps", bufs=2, space="PSUM") as f1ps:
            for fM in range(FT):
                w1t = f1p.tile([P, HT, P], F32, tag="w1t")
                nc.sync.dma_start(r(w1t[:]), r(w1_t[fM]))
                ps = f1ps.tile([P, QR], F32, tag="f1psum")
                for ht in range(HT):
                    nc.tensor.matmul(ps[:], r(w1t[:, ht, :]),
                                     r(g_res[:, ht, :]),
                                     start=(ht == 0), stop=(ht == HT - 1))
                nc.scalar.activation(r(f_res[:, fM, :]), ps[:], AF.Gelu,
                                     bias=b1_t[:, fM:fM + 1])
        cm_g.__exit__(None, None, None)

        # ============ S10: FFN2 + bias + residual -> output ================
        with tc.tile_pool(name="ffn2", bufs=2) as f2p, \
             tc.tile_pool(name="f2ps", bufs=2, space="PSUM") as f2ps:
            for dM in range(HT):
                ps = f2ps.tile([P, QR], F32, tag="f2psum")
                for q4 in range(4):
                    w2t = f2p.tile([P, HT, P], F32, tag="w2t")
                    nc.sync.dma_start(
                        r(w2t[:]), r(w2_t[dM][:, 16 * q4:16 * (q4 + 1), :]))
                    for ft in range(HT):
                        kk = 16 * q4 + ft
                        nc.tensor.matmul(ps[:], r(w2t[:, ft, :]),
                                         r(f_res[:, kk, :]),
                                         start=(kk == 0), stop=(kk == FT - 1))
                hfin = f2p.tile([P, QR], F32, tag="hfin")
                nc.sync.dma_start(hfin[:], h_d[P * dM:P * (dM + 1), :])
                ost = f2p.tile([P, QR], F32, tag="ost")
                nc.vector.scalar_tensor_tensor(
                    ost[:], ps[:], b2_t[:, dM:dM + 1], hfin[:],
                    op0=OP.add, op1=OP.add)
                nc.sync.dma_start(outT[P * dM:P * (dM + 1), :], ost[:])
        cm_f.__exit__(None, None, None)
        cm_dram.__exit__(None, None, None)
        cm_const.__exit__(None, None, None)

    return nc


# ---------------------------------------------------------------------------
# Host side
# ---------------------------------------------------------------------------

_CACHE = {}


def _get_nc(debug_outputs=()):
    key = tuple(sorted(debug_outputs))
    if key not in _CACHE:
        _CACHE[key] = build_nc(debug_outputs)
    return _CACHE[key]


def make_in_maps(inputs):
    x = np.asarray(inputs["x"], np.float32)
    scale = np.float32(1.0 / np.sqrt(HD))
    wqkv = np.asarray(inputs["wqkv_w"], np.float32)
    wqkv_b = np.asarray(inputs["wqkv_b"], np.float32)
    def tile_kxm(wT):
        # [K, M] -> [mM, p, kt, m2] so each [128, kt*128] lhsT load is
        # contiguous per partition
        K_, M_ = wT.shape
        return np.ascontiguousarray(
            wT.reshape(K_ // P, P, M_ // P, P).transpose(2, 1, 0, 3))

    shared = {
        "ones_d": np.ones((P, 1), np.float32),
        "ones_r_d": np.ones((1, P), np.float32),
        "wq_t": tile_kxm(wqkv[:H].T * scale),
        "wk_t": tile_kxm(np.ascontiguousarray(wqkv[H:2 * H].T)),
        "wvT": np.ascontiguousarray(wqkv[2 * H:].T),
        "wo_t": tile_kxm(np.asarray(inputs["wo_w"], np.float32).T),
        "w1_t": tile_kxm(np.asarray(inputs["w1"], np.float32).T),
        "w2_t": tile_kxm(np.asarray(inputs["w2"], np.float32).T),
        "bq": np.ascontiguousarray(wqkv_b[:H] * scale),
        "bk": np.ascontiguousarray(wqkv_b[H:2 * H]),
        "bv": np.ascontiguousarray(wqkv_b[2 * H:]),
        "bwo": np.asarray(inputs["wo_b"], np.float32),
        "b1": np.asarray(inputs["b1"], np.float32),
        "b2": np.asarray(inputs["b2"], np.float32),
        "ln1w": np.asarray(inputs["ln1_w"], np.float32),
        "ln1b": np.asarray(inputs["ln1_b"], np.float32),
        "ln2w": np.asarray(inputs["ln2_w"], np.float32),
        "ln2b": np.asarray(inputs["ln2_b"], np.float32),
    }
    kidx = np.arange(S)
    in_maps = []
    for core in range(8):
        b, c = divmod(core, 4)
        q0 = QR * c
        qidx = q0 + np.arange(QR)
        m = np.where(kidx[:, None] <= qidx[None, :], np.float32(0),
                     np.float32(NEG)).astype(np.float32)
        in_maps.append(dict(
            shared,
            xT=np.ascontiguousarray(x[b].T),
            xTq=np.ascontiguousarray(x[b, q0:q0 + QR].T),
            maskT=np.ascontiguousarray(m),
        ))
    return in_maps


def run_cores(inputs, debug_outputs=(), **run_kw):
    nc = _get_nc(debug_outputs)
    in_maps = make_in_maps(inputs)
    return nc, run_bass_kernel_spmd(nc, in_maps, core_ids=list(range(8)),
                                    **run_kw)


def kernel(**inputs):
    _, res = run_cores(inputs)
    out = np.empty((B, S, H), np.float32)
    for core in range(8):
        b, c = divmod(core, 4)
        out[b, QR * c:QR * (c + 1), :] = res.results[core]["outT"].T
    return out



# revision 2
# speedup vs baseline: 1.2085x; 1.2085x over previous
"""Trainium2 Bass kernel v2 for the dense transformer decoder block.

Problem: B=2, S=2048, H=2048, NH=16 (head_dim=128), FFN=8192, fp32 in/out.

Sharding: head-parallel attention + query-parallel FFN, stitched by two
1MB AllToAlls.  Core g computes LN1 + Q/K/V projections for heads
{g, g+8} over BOTH batches (no redundant K/V work), runs causally-tiled
attention for those heads (upper-triangle tiles skipped; only
diagonal-crossing tiles pay a 0/1-mask multiply), then an 8-core
AllToAll redistributes the attention output so core g holds ALL 2048
features for its 512-query block (b=g//4, rows 512*(g%4)...).  WO, LN2
and the FFN then run on that block exactly once per core.

All matmuls run in bf16 (fp32 PSUM accumulation): same PE stream rate
as f32r but half the DMA/SBUF and 2x faster weight loads; rel-err
budget (2e-2) has plenty of headroom.  LayerNorm/softmax partition-dim
statistics use ones-vector matmuls; softmax denominators for a head are
batched into a single [8,512] vector reciprocal.
"""

import json

import numpy as np
import ml_dtypes

import concourse.bass as bass
import concourse.bass2jax as bass2jax
import concourse.mybir as mybir
import concourse.tile as tile
from concourse.bass_utils import compile_bir_kernel as _orig_compile_bir_kernel
from concourse.bass_utils import run_bass_kernel_spmd

F32 = mybir.dt.float32
BF16 = mybir.dt.bfloat16
AF = mybir.ActivationFunctionType
OP = mybir.AluOpType
NPBF = ml_dtypes.bfloat16

B, S, H, NH, HD, FF = 2, 2048, 2048, 16, 128, 8192
P = 128
QR = 512            # query rows per core in the FFN phase
HT = H // P         # 16 feature tiles
FT = FF // P        # 64 ffn tiles
NKT = S // P        # 16 key tiles per batch
EPS = 1e-5

# ---------------------------------------------------------------------------
# Workaround for this container's walrus build: it supports only ONE sync
# wait per instruction, but Tile attaches several.  Rewrite the BIR just
# before walrus: an instruction with N>1 waits gets N-1 same-engine NoOps
# inserted before it, each carrying one wait.
# ---------------------------------------------------------------------------


def _split_multiwaits(bir_bytes):
    bir = json.loads(bir_bytes)
    ctr = 0
    for fn in bir.get("functions", []):
        for blk in fn.get("blocks", []):
            new = []
            for inst in blk.get("instructions", []):
                si = inst.get("sync_info")
                waits = (si or {}).get("on_wait") or []
                if len(waits) > 1:
                    for w in waits[:-1]:
                        ctr += 1
                        new.append({
                            "engine": inst["engine"],
                            "ins": [],
                            "outs": [],
                            "name": f"I-mwsplit{ctr}",
                            "opcode": "NoOp",
                            "sync_info": {"on_update": [], "on_wait": [w]},
                            "text_hint": "multiwait_split",
                        })
                    si["on_wait"] = [waits[-1]]
                new.append(inst)
            blk["instructions"] = new
    return json.dumps(bir).encode()


def _patched_compile_bir_kernel(bir_json, tmpdir, neff_name="file.neff", **kw):
    if isinstance(bir_json, str):
        bir_json = bir_json.encode()
    return _orig_compile_bir_kernel(_split_multiwaits(bir_json), tmpdir,
                                    neff_name=neff_name, **kw)


def _install_patch():
    bass2jax.compile_bir_kernel = _patched_compile_bir_kernel


# ---------------------------------------------------------------------------
# Device program
# ---------------------------------------------------------------------------


def _ln_stats_rows(nc, tc, pool, ones_bf, load, n, tag):
    """Feature-dim (partition) LN stats over HT tiles of [P, n] bf16.

    Emits the stats matmuls (max 4 PSUM banks: processes 512-col chunks
    two at a time) and the row math.  Returns (rowm, rowr): [1, n] bf16
    rows of mean and rstd.
    """
    nch = n // 512
    mean4 = pool.tile([P, 512], F32, tag=f"{tag}_m4", bufs=1)
    msq4 = pool.tile([P, 512], F32, tag=f"{tag}_q4", bufs=1)
    with tc.tile_pool(name=f"{tag}_sps", bufs=1, space="PSUM") as sps:
        for h0 in range(0, nch, 2):
            hn = min(2, nch - h0)
            mps = [sps.tile([1, 512], F32, tag=f"{tag}_mps{c}",
                            name=f"{tag}_mps{h0}_{c}") for c in range(hn)]
            qps = [sps.tile([1, 512], F32, tag=f"{tag}_qps{c}",
                            name=f"{tag}_qps{h0}_{c}") for c in range(hn)]
            for i in range(HT):
                xt = load(i)
                xsq = pool.tile([P, 512 * hn], BF16, tag=f"{tag}_sq", bufs=1)
                nc.vector.tensor_mul(
                    xsq[:], xt[:, 512 * h0:512 * (h0 + hn)],
                    xt[:, 512 * h0:512 * (h0 + hn)])
                for c in range(hn):
                    sl = slice(512 * (h0 + c), 512 * (h0 + c + 1))
                    nc.tensor.matmul(mps[c][:], ones_bf[:], xt[:, sl],
                                     start=(i == 0), stop=(i == HT - 1))
                    nc.tensor.matmul(qps[c][:], ones_bf[:],
                                     xsq[:, 512 * c:512 * (c + 1)],
                                     start=(i == 0), stop=(i == HT - 1))
            for c in range(hn):
                r = 32 * (h0 + c)
                nc.scalar.activation(mean4[r:r + 1, :], mps[c][:], AF.Copy,
                                     scale=1.0 / H)
                nc.scalar.activation(msq4[r:r + 1, :], qps[c][:], AF.Copy,
                                     scale=1.0 / H)
    var = pool.tile([P, 512], F32, tag=f"{tag}_var", bufs=1)
    rstd = pool.tile([P, 512], F32, tag=f"{tag}_rstd", bufs=1)
    nc.vector.tensor_mul(var[:], mean4[:], mean4[:])
    nc.vector.tensor_sub(var[:], msq4[:], var[:])
    nc.vector.tensor_scalar_add(var[:], var[:], EPS)
    nc.vector.reciprocal(var[:], var[:])
    nc.scalar.activation(rstd[:], var[:], AF.Sqrt)
    rowm = pool.tile([1, n], BF16, tag=f"{tag}_rowm", bufs=1)
    rowr = pool.tile([1, n], BF16, tag=f"{tag}_rowr", bufs=1)
    for r in range(nch):
        sl = slice(512 * r, 512 * (r + 1))
        nc.scalar.activation(rowm[0:1, sl], mean4[32 * r:32 * r + 1, :],
                             AF.Copy)
        nc.scalar.activation(rowr[0:1, sl], rstd[32 * r:32 * r + 1, :],
                             AF.Copy)
    return rowm, rowr


def _ln_bcast(nc, tc, pool, ones_row_bf, rowm, rowr, n, tag):
    """Broadcast [1, n] mean/rstd rows to [P, n] bf16 tiles via K=1 MMs."""
    nch = n // 512
    bmean = pool.tile([P, n], BF16, tag=f"{tag}_bm", bufs=1)
    brstd = pool.tile([P, n], BF16, tag=f"{tag}_br", bufs=1)
    with tc.tile_pool(name=f"{tag}_bps", bufs=2, space="PSUM") as bps:
        for r in range(nch):
            sl = slice(512 * r, 512 * (r + 1))
            mp = bps.tile([P, 512], F32, tag=f"{tag}_bmp", name=f"{tag}_bmp{r}")
            nc.tensor.matmul(mp[:], ones_row_bf[:], rowm[0:1, sl],
                             start=True, stop=True)
            nc.scalar.activation(bmean[:, sl], mp[:], AF.Copy)
            rp = bps.tile([P, 512], F32, tag=f"{tag}_brp", name=f"{tag}_brp{r}")
            nc.tensor.matmul(rp[:], ones_row_bf[:], rowr[0:1, sl],
                             start=True, stop=True)
            nc.scalar.activation(brstd[:, sl], rp[:], AF.Copy)
    return bmean, brstd


def build_nc(debug_outputs=()):
    _install_patch()
    nc = bass.Bass("TRN2")

    xT_t = nc.dram_tensor("xT_t", (B, P, HT, S), BF16, kind="ExternalInput")
    xq32 = nc.dram_tensor("xq32", (P, HT, QR), F32, kind="ExternalInput")
    wq_h = nc.dram_tensor("wq_h", (P, 2, HT, P), BF16, kind="ExternalInput")
    wk_h = nc.dram_tensor("wk_h", (P, 2, HT, P), BF16, kind="ExternalInput")
    wvT_h = nc.dram_tensor("wvT_h", (P, HT, 2 * P), BF16, kind="ExternalInput")
    wo_h = nc.dram_tensor("wo_h", (HT, P, HT, P), BF16, kind="ExternalInput")
    w1_h = nc.dram_tensor("w1_h", (FT, P, HT, P), BF16, kind="ExternalInput")
    w2_h = nc.dram_tensor("w2_h", (HT, P, FT, P), BF16, kind="ExternalInput")
    mask_h = nc.dram_tensor("mask_h", (P, 4, QR), BF16, kind="ExternalInput")
    ones_d = nc.dram_tensor("ones_d", (P, 1), BF16, kind="ExternalInput")
    ones_r_d = nc.dram_tensor("ones_r_d", (1, P), BF16, kind="ExternalInput")
    bq_d = nc.dram_tensor("bq_d", (P, 2), F32, kind="ExternalInput")
    bk_d = nc.dram_tensor("bk_d", (P, 2), F32, kind="ExternalInput")
    bv_d = nc.dram_tensor("bv_d", (P, 2), F32, kind="ExternalInput")
    bwo_d = nc.dram_tensor("bwo_d", (P, HT), F32, kind="ExternalInput")
    b1_d = nc.dram_tensor("b1_d", (P, FT), F32, kind="ExternalInput")
    b2_d = nc.dram_tensor("b2_d", (P, HT), F32, kind="ExternalInput")
    ln1w_d = nc.dram_tensor("ln1w_d", (P, HT), F32, kind="ExternalInput")
    ln1b_d = nc.dram_tensor("ln1b_d", (P, HT), F32, kind="ExternalInput")
    ln2w_d = nc.dram_tensor("ln2w_d", (P, HT), F32, kind="ExternalInput")
    ln2b_d = nc.dram_tensor("ln2b_d", (P, HT), F32, kind="ExternalInput")
    outT = nc.dram_tensor("outT", (H, QR), F32, kind="ExternalOutput")

    a2a_in = [nc.dram_tensor(f"a2a_in{i}", (8, P, QR), BF16) for i in range(2)]
    a2a_out = [nc.dram_tensor(f"a2a_out{i}", (8, P, QR), BF16)
               for i in range(2)]

    dbg = {}
    for name, shape, dt in (("q0", (P, 2, S), BF16), ("k0", (P, 2, S), BF16),
                            ("v0", (P, NKT, 2 * P), BF16),
                            ("av0", (P, 8, QR), BF16), ("av1", (P, 8, QR), BF16),
                            ("rx", (P, HT, QR), BF16),
                            ("h", (P, HT, QR), BF16), ("g", (P, HT, QR), BF16)):
        if name in debug_outputs:
            dbg[name] = nc.dram_tensor(f"dbg_{name}", shape, dt,
                                       kind="ExternalOutput")

    with tile.TileContext(nc) as tc:
        cm_const = tc.tile_pool(name="const", bufs=1)
        const = cm_const.__enter__()
        ones_bf = const.tile([P, 1], BF16, tag="ones")
        nc.sync.dma_start(ones_bf[:], ones_d[:])
        ones_row_bf = const.tile([1, P], BF16, tag="ones_row")
        nc.sync.dma_start(ones_row_bf[:], ones_r_d[:])
        mask_sb = const.tile([P, 4, QR], BF16, tag="mask")
        nc.sync.dma_start(mask_sb[:], mask_h[:])
        wq_sb = const.tile([P, 2, HT, P], BF16, tag="wq")
        nc.sync.dma_start(wq_sb[:], wq_h[:])
        wk_sb = const.tile([P, 2, HT, P], BF16, tag="wk")
        nc.sync.dma_start(wk_sb[:], wk_h[:])
        wvT_sb = const.tile([P, HT, 2 * P], BF16, tag="wv")
        nc.sync.dma_start(wvT_sb[:], wvT_h[:])

        def bias_tile(name, dram_t, ntiles):
            t = const.tile([P, ntiles], F32, tag=f"b_{name}")
            nc.sync.dma_start(t[:], dram_t[:])
            return t

        bq_t = bias_tile("bq", bq_d, 2)
        bk_t = bias_tile("bk", bk_d, 2)
        bv_t = bias_tile("bv", bv_d, 2)
        bwo_t = bias_tile("bwo", bwo_d, HT)
        b1_t = bias_tile("b1", b1_d, FT)
        b2_t = bias_tile("b2", b2_d, HT)
        ln1w_t = bias_tile("ln1w", ln1w_d, HT)
        ln1b_t = bias_tile("ln1b", ln1b_d, HT)
        ln2w_t = bias_tile("ln2w", ln2w_d, HT)
        ln2b_t = bias_tile("ln2b", ln2b_d, HT)

        # Persistent per-batch Q/K/V results and attention outputs.
        cm_qkv = tc.tile_pool(name="qkv", bufs=1)
        qkvp = cm_qkv.__enter__()
        q_sb = [qkvp.tile([P, 2, S], BF16, tag=f"q{b}", name=f"q{b}")
                for b in range(B)]
        k_sb = [qkvp.tile([P, 2, S], BF16, tag=f"k{b}", name=f"k{b}")
                for b in range(B)]
        vT_sb = [qkvp.tile([P, NKT, 2 * P], BF16, tag=f"v{b}", name=f"v{b}")
                 for b in range(B)]

        cm_av = tc.tile_pool(name="av", bufs=1)
        avp_ = cm_av.__enter__()
        av_keep = [avp_.tile([P, 8, QR], BF16, tag=f"avk{h}", name=f"avk{h}")
                   for h in range(2)]
        dn_all = [[avp_.tile([P, QR], F32, tag=f"dn{h}{b}",
                              name=f"dnall{h}{b}") for b in range(B)]
                  for h in range(2)]

        x_pools = {}

        def phase_a_stats(b):
            cm = tc.tile_pool(name=f"x{b}", bufs=1)
            xp = cm.__enter__()
            x_sb = xp.tile([P, HT, S], BF16, tag=f"x{b}")
            for i in range(HT):
                nc.sync.dma_start(x_sb[:, i, :], xT_t[b, :, i, :])
            lp_cm = tc.tile_pool(name=f"ln1_{b}", bufs=1)
            lp = lp_cm.__enter__()
            rowm, rowr = _ln_stats_rows(nc, tc, lp, ones_bf,
                                        lambda i: x_sb[:, i, :], S, f"l1{b}")
            x_pools[b] = (cm, xp, x_sb, lp_cm, lp, rowm, rowr)

        def phase_a_apply_qkv(b):
            cm, xp, x_sb, lp_cm, lp, rowm, rowr = x_pools[b]
            bmean, brstd = _ln_bcast(nc, tc, lp, ones_row_bf, rowm, rowr,
                                     S, f"l1{b}")
            with tc.tile_pool(name=f"prj{b}", bufs=1, space="PSUM") as pps:
                for c4 in range(4):
                    csl = slice(QR * c4, QR * (c4 + 1))
                    for i in range(HT):
                        t1 = lp.tile([P, QR], BF16, tag="t1", bufs=2)
                        nc.vector.tensor_sub(t1[:], x_sb[:, i, csl],
                                             bmean[:, csl])
                        t2 = lp.tile([P, QR], BF16, tag="t2", bufs=2)
                        nc.vector.tensor_mul(t2[:], t1[:], brstd[:, csl])
                        nc.scalar.activation(x_sb[:, i, csl], t2[:],
                                             AF.Identity,
                                             bias=ln1b_t[:, i:i + 1],
                                             scale=ln1w_t[:, i:i + 1])
                    for w_sb, bias_t, dst in ((wq_sb, bq_t, q_sb[b]),
                                              (wk_sb, bk_t, k_sb[b])):
                        for m in range(2):
                            ps = pps.tile([P, QR], F32, tag="qkps", bufs=2,
                                          name=f"qk{b}_{c4}_{id(w_sb)}_{m}")
                            for ht in range(HT):
                                nc.tensor.matmul(
                                    ps[:], w_sb[:, m, ht, :],
                                    x_sb[:, ht, csl],
                                    start=(ht == 0), stop=(ht == HT - 1))
                            nc.vector.tensor_scalar_add(
                                dst[:, m, csl], ps[:], bias_t[:, m:m + 1])
                    for sM in range(4 * c4, 4 * c4 + 4):
                        vp = pps.tile([P, 2 * P], F32, tag="vps", bufs=2,
                                      name=f"v{b}_{sM}")
                        for ht in range(HT):
                            nc.tensor.matmul(vp[:],
                                             x_sb[:, ht, P * sM:P * (sM + 1)],
                                             wvT_sb[:, ht, :],
                                             start=(ht == 0),
                                             stop=(ht == HT - 1))
                        nc.vector.tensor_scalar_add(vT_sb[b][:, sM, :], vp[:],
                                                    0.0)
            lp_cm.__exit__(None, None, None)
            cm.__exit__(None, None, None)

        def attn_hb(hd_t, b, ap_, aps):
            for qb in range(4):
                nk = 4 * qb + 4
                pt = ap_.tile([P, NKT, QR], BF16, tag="pt", bufs=1)
                for k2 in range(0, nk, 2):
                    sp2 = aps.tile([P, 2, QR], F32, tag="sp2", bufs=2,
                                   name=f"sp{hd_t}_{b}_{qb}_{k2}")
                    for j in range(2):
                        kt = k2 + j
                        nc.tensor.matmul(
                            sp2[:, j, :],
                            k_sb[b][:, hd_t, P * kt:P * (kt + 1)],
                            q_sb[b][:, hd_t, QR * qb:QR * (qb + 1)],
                            start=True, stop=True)
                    nc.scalar.activation(pt[:, k2:k2 + 2, :], sp2[:], AF.Exp)
                    if k2 >= 4 * qb:
                        j0 = k2 - 4 * qb
                        nc.vector.tensor_mul(pt[:, k2:k2 + 2, :],
                                             pt[:, k2:k2 + 2, :],
                                             mask_sb[:, j0:j0 + 2, :])
                r = 4 * b + qb
                dnp = aps.tile([1, QR], F32, tag="dn", bufs=1)
                for kt in range(nk):
                    nc.tensor.matmul(dnp[:], ones_bf[:], pt[:, kt, :],
                                     start=(kt == 0), stop=(kt == nk - 1))
                nc.scalar.activation(dn_all[hd_t][b][32 * qb:32 * qb + 1, :],
                                     dnp[:], AF.Copy)
                avp = aps.tile([P, QR], F32, tag="av", bufs=1)
                for kt in range(nk):
                    nc.tensor.matmul(avp[:],
                                     vT_sb[b][:, kt, P * hd_t:P * (hd_t + 1)],
                                     pt[:, kt, :],
                                     start=(kt == 0), stop=(kt == nk - 1))
                nc.scalar.activation(av_keep[hd_t][:, r, :], avp[:], AF.Copy)

        def norm_a2a(hd_t, np_):
            rrow = np_.tile([1, 8 * QR], BF16, tag=f"rrow{hd_t}", bufs=1)
            for b in range(B):
                rec = np_.tile([P, QR], F32, tag="rec", bufs=1)
                nc.vector.reciprocal(rec[:], dn_all[hd_t][b][:])
                recbf = np_.tile([P, QR], BF16, tag="recbf", bufs=1)
                nc.scalar.activation(recbf[:], rec[:], AF.Copy)
                for qb in range(4):
                    r = 4 * b + qb
                    nc.scalar.activation(rrow[0:1, QR * r:QR * (r + 1)],
                                         recbf[32 * qb:32 * qb + 1, :],
                                         AF.Copy)
            with tc.tile_pool(name=f"nps{hd_t}", bufs=2, space="PSUM") as nps:
                for r in range(8):
                    bp = nps.tile([P, QR], F32, tag="brec", name=f"brc{hd_t}{r}")
                    nc.tensor.matmul(bp[:], ones_row_bf[:],
                                     rrow[0:1, QR * r:QR * (r + 1)],
                                     start=True, stop=True)
                    brec = np_.tile([P, QR], BF16, tag="brecs", bufs=1)
                    nc.scalar.activation(brec[:], bp[:], AF.Copy)
                    st = np_.tile([P, QR], BF16, tag="avst", bufs=1)
                    nc.vector.tensor_mul(st[:], av_keep[hd_t][:, r, :], brec[:])
                    nc.vector.tensor_scalar_add(st[:], st[:],
                                                bv_t[:, hd_t:hd_t + 1])
                    nc.sync.dma_start(a2a_in[hd_t][r], st[:])
            nc.gpsimd.collective_compute(
                "AllToAll", OP.bypass, replica_groups=[list(range(8))],
                ins=[a2a_in[hd_t][:].opt()], outs=[a2a_out[hd_t][:].opt()])

        # ---- schedule ----
        phase_a_stats(0)
        phase_a_apply_qkv(0)
        phase_a_stats(1)
        phase_a_apply_qkv(1)
        cm_attn = tc.tile_pool(name="attn", bufs=1)
        atp = cm_attn.__enter__()
        cm_aps = tc.tile_pool(name="attnps", bufs=2, space="PSUM")
        aps = cm_aps.__enter__()
        attn_hb(0, 0, atp, aps)
        attn_hb(0, 1, atp, aps)
        cm_norm = tc.tile_pool(name="norm", bufs=1)
        np_ = cm_norm.__enter__()
        norm_a2a(0, np_)
        attn_hb(1, 0, atp, aps)
        attn_hb(1, 1, atp, aps)
        norm_a2a(1, np_)

        if "q0" in dbg:
            nc.sync.dma_start(dbg["q0"][:], q_sb[0][:])
        if "k0" in dbg:
            nc.sync.dma_start(dbg["k0"][:], k_sb[0][:])
        if "v0" in dbg:
            nc.sync.dma_start(dbg["v0"][:], vT_sb[0][:])
        for hh in range(2):
            if f"av{hh}" in dbg:
                nc.sync.dma_start(dbg[f"av{hh}"][:], av_keep[hh][:])

        cm_norm.__exit__(None, None, None)
        cm_aps.__exit__(None, None, None)
        cm_attn.__exit__(None, None, None)
        cm_av.__exit__(None, None, None)
        cm_qkv.__exit__(None, None, None)

        # ---- phase C: WO + residual, LN2, FFN ----
        cm_c = tc.tile_pool(name="cres", bufs=1)
        cp = cm_c.__enter__()
        rx = cp.tile([P, HT, QR], BF16, tag="rx")
        nc.sync.dma_start(rx[:, 0:8, :], a2a_out[0].rearrange("j p q -> p j q"))
        nc.sync.dma_start(rx[:, 8:16, :],
                          a2a_out[1].rearrange("j p q -> p j q"))
        if "rx" in dbg:
            nc.sync.dma_start(dbg["rx"][:], rx[:])
        h_bf = cp.tile([P, HT, QR], BF16, tag="hbf")
        woA = cp.tile([P, HT, QR], F32, tag="woA")
        with tc.tile_pool(name="wo", bufs=2) as wop, \
             tc.tile_pool(name="wops", bufs=2, space="PSUM") as wops:
            for dM in range(HT):
                wot = wop.tile([P, 8, P], BF16, tag="wotA")
                nc.sync.dma_start(wot[:], wo_h[dM][:, 0:8, :])
                ps = wops.tile([P, QR], F32, tag="wops")
                for kt in range(8):
                    nc.tensor.matmul(ps[:], wot[:, kt, :], rx[:, kt, :],
                                     start=(kt == 0), stop=(kt == 7))
                nc.scalar.activation(woA[:, dM, :], ps[:], AF.Copy)
            for dM in range(HT):
                wot = wop.tile([P, 8, P], BF16, tag="wotB")
                nc.sync.dma_start(wot[:], wo_h[dM][:, 8:16, :])
                xq_t = wop.tile([P, QR], F32, tag="xqt")
                nc.sync.dma_start(xq_t[:], xq32[:, dM, :])
                ps = wops.tile([P, QR], F32, tag="wops")
                for kt in range(8):
                    nc.tensor.matmul(ps[:], wot[:, kt, :], rx[:, 8 + kt, :],
                                     start=(kt == 0), stop=(kt == 7))
                tsum = wop.tile([P, QR], F32, tag="tsum")
                nc.vector.tensor_add(tsum[:], ps[:], woA[:, dM, :])
                nc.vector.scalar_tensor_tensor(
                    h_bf[:, dM, :], tsum[:], bwo_t[:, dM:dM + 1],
                    xq_t[:], op0=OP.add, op1=OP.add)
        if "h" in dbg:
            nc.sync.dma_start(dbg["h"][:], h_bf[:])

        g_sb = cp.tile([P, HT, QR], BF16, tag="g")
        with tc.tile_pool(name="ln2", bufs=1) as l2p:
            rowm2, rowr2 = _ln_stats_rows(nc, tc, l2p, ones_bf,
                                          lambda i: h_bf[:, i, :], QR, "l2")
            bmean2, brstd2 = _ln_bcast(nc, tc, l2p, ones_row_bf, rowm2, rowr2,
                                       QR, "l2")
            for i in range(HT):
                t1 = l2p.tile([P, QR], BF16, tag="t1", bufs=2)
                nc.vector.tensor_sub(t1[:], h_bf[:, i, :], bmean2[:])
                t2 = l2p.tile([P, QR], BF16, tag="t2", bufs=2)
                nc.vector.tensor_mul(t2[:], t1[:], brstd2[:])
                nc.scalar.activation(g_sb[:, i, :], t2[:], AF.Identity,
                                     bias=ln2b_t[:, i:i + 1],
                                     scale=ln2w_t[:, i:i + 1])
        if "g" in dbg:
            nc.sync.dma_start(dbg["g"][:], g_sb[:])

        f_sb = cp.tile([P, FT, QR], BF16, tag="f")
        with tc.tile_pool(name="ffn1", bufs=3) as f1p, \
             tc.tile_pool(name="f1ps", bufs=2, space="PSUM") as f1ps:
            for fM in range(FT):
                w1t = f1p.tile([P, HT, P], BF16, tag="w1t")
                nc.sync.dma_start(w1t[:], w1_h[fM])
                ps = f1ps.tile([P, QR], F32, tag="f1ps")
                for ht in range(HT):
                    nc.tensor.matmul(ps[:], w1t[:, ht, :], g_sb[:, ht, :],
                                     start=(ht == 0), stop=(ht == HT - 1))
                nc.scalar.activation(f_sb[:, fM, :], ps[:], AF.Gelu,
                                     bias=b1_t[:, fM:fM + 1])

        with tc.tile_pool(name="ffn2", bufs=2) as f2p, \
             tc.tile_pool(name="f2ps", bufs=2, space="PSUM") as f2ps:
            for dM in range(HT):
                ps = f2ps.tile([P, QR], F32, tag="f2ps")
                for q4 in range(4):
                    w2t = f2p.tile([P, HT, P], BF16, tag="w2t")
                    nc.sync.dma_start(w2t[:], w2_h[dM][:, HT * q4:HT * (q4 + 1), :])
                    for ft in range(HT):
                        kk = HT * q4 + ft
                        nc.tensor.matmul(ps[:], w2t[:, ft, :], f_sb[:, kk, :],
                                         start=(kk == 0), stop=(kk == FT - 1))
                ost = f2p.tile([P, QR], F32, tag="ost")
                nc.vector.scalar_tensor_tensor(
                    ost[:], ps[:], b2_t[:, dM:dM + 1], h_bf[:, dM, :],
                    op0=OP.add, op1=OP.add)
                nc.sync.dma_start(outT[P * dM:P * (dM + 1), :], ost[:])
        cm_c.__exit__(None, None, None)
        cm_const.__exit__(None, None, None)

    return nc


# ---------------------------------------------------------------------------
# Host side
# ---------------------------------------------------------------------------

_CACHE = {}


def _get_nc(debug_outputs=()):
    key = tuple(sorted(debug_outputs))
    if key not in _CACHE:
        _CACHE[key] = build_nc(debug_outputs)
    return _CACHE[key]


def tile_kxm(wT):
    # [K, M] -> [mM, p(K), kt, m2] so each [128, kt*128] lhsT load is
    # contiguous per partition.
    K_, M_ = wT.shape
    return np.ascontiguousarray(
        wT.reshape(K_ // P, P, M_ // P, P).transpose(2, 1, 0, 3))


def pcol(v):
    # [n*P] -> [P, n] bias tile (column m holds elements m*P..m*P+P-1)
    return np.ascontiguousarray(
        np.asarray(v, np.float32).reshape(-1, P).T)


def make_in_maps(inputs):
    x = np.asarray(inputs["x"], np.float32)
    scale = np.float32(1.0 / np.sqrt(HD))
    wqkv = np.asarray(inputs["wqkv_w"], np.float32)
    wqkv_b = np.asarray(inputs["wqkv_b"], np.float32)

    xT_t = np.ascontiguousarray(
        x.transpose(0, 2, 1).reshape(B, HT, P, S).transpose(0, 2, 1, 3)
    ).astype(NPBF)

    mask = np.zeros((P, 4, QR), np.float32)
    r_ = np.arange(P)[:, None]
    c_ = np.arange(QR)[None, :]
    for j in range(4):
        mask[:, j, :] = (P * j + r_ <= c_).astype(np.float32)

    shared = {
        "xT_t": xT_t,
        "mask_h": mask.astype(NPBF),
        "ones_d": np.ones((P, 1), NPBF),
        "ones_r_d": np.ones((1, P), NPBF),
        "wo_h": tile_kxm(np.asarray(inputs["wo_w"], np.float32).T).astype(NPBF),
        "w1_h": tile_kxm(np.asarray(inputs["w1"], np.float32).T).astype(NPBF),
        "w2_h": tile_kxm(np.asarray(inputs["w2"], np.float32).T).astype(NPBF),
        "bwo_d": pcol(inputs["wo_b"]),
        "b1_d": pcol(inputs["b1"]),
        "b2_d": pcol(inputs["b2"]),
        "ln1w_d": pcol(inputs["ln1_w"]),
        "ln1b_d": pcol(inputs["ln1_b"]),
        "ln2w_d": pcol(inputs["ln2_w"]),
        "ln2b_d": pcol(inputs["ln2_b"]),
    }
    in_maps = []
    for core in range(8):
        rows = np.r_[core * P:(core + 1) * P,
                     (core + 8) * P:(core + 9) * P]
        wq_s = wqkv[:H][rows] * scale
        wk_s = wqkv[H:2 * H][rows]
        wv_s = wqkv[2 * H:][rows]
        b_out, c_out = divmod(core, 4)
        xq = x[b_out, QR * c_out:QR * (c_out + 1), :].T  # [H, QR]
        in_maps.append(dict(
            shared,
            xq32=np.ascontiguousarray(
                xq.reshape(HT, P, QR).transpose(1, 0, 2)),
            wq_h=np.ascontiguousarray(
                tile_kxm(wq_s.T).transpose(1, 0, 2, 3)).astype(NPBF),
            wk_h=np.ascontiguousarray(
                tile_kxm(wk_s.T).transpose(1, 0, 2, 3)).astype(NPBF),
            wvT_h=np.ascontiguousarray(
                wv_s.T.reshape(HT, P, 2 * P).transpose(1, 0, 2)).astype(NPBF),
            bq_d=np.ascontiguousarray(
                (wqkv_b[:H][rows] * scale).reshape(2, P).T),
            bk_d=np.ascontiguousarray(wqkv_b[H:2 * H][rows].reshape(2, P).T),
            bv_d=np.ascontiguousarray(wqkv_b[2 * H:][rows].reshape(2, P).T),
        ))
    return in_maps


def run_cores(inputs, debug_outputs=(), **run_kw):
    nc = _get_nc(debug_outputs)
    in_maps = make_in_maps(inputs)
    return nc, run_bass_kernel_spmd(nc, in_maps, core_ids=list(range(8)),
                                    **run_kw)


def kernel(**inputs):
    _, res = run_cores(inputs)
    out = np.empty((B, S, H), np.float32)
    for core in range(8):
        b, c = divmod(core, 4)
        out[b, QR * c:QR * (c + 1), :] = res.results[core]["outT"].T
    return out


# revision 3
# speedup vs baseline: 1.2297x; 1.0176x over previous
"""Trainium2 Bass kernel v2 for the dense transformer decoder block.

Problem: B=2, S=2048, H=2048, NH=16 (head_dim=128), FFN=8192, fp32 in/out.

Sharding: head-parallel attention + query-parallel FFN, stitched by two
1MB AllToAlls.  Core g computes LN1 + Q/K/V projections for heads
{g, g+8} over BOTH batches (no redundant K/V work), runs causally-tiled
attention for those heads (upper-triangle tiles skipped; only
diagonal-crossing tiles pay a 0/1-mask multiply), then an 8-core
AllToAll redistributes the attention output so core g holds ALL 2048
features for its 512-query block (b=g//4, rows 512*(g%4)...).  WO, LN2
and the FFN then run on that block exactly once per core.

All matmuls run in bf16 (fp32 PSUM accumulation): same PE stream rate
as f32r but half the DMA/SBUF and 2x faster weight loads; rel-err
budget (2e-2) has plenty of headroom.  LayerNorm/softmax partition-dim
statistics use ones-vector matmuls; softmax denominators for a head are
batched into a single [8,512] vector reciprocal.
"""

import json

import numpy as np
import ml_dtypes

import concourse.bass as bass
import concourse.bass2jax as bass2jax
import concourse.mybir as mybir
import concourse.tile as tile
from concourse.bass_utils import compile_bir_kernel as _orig_compile_bir_kernel
from concourse.bass_utils import run_bass_kernel_spmd

F32 = mybir.dt.float32
BF16 = mybir.dt.bfloat16
AF = mybir.ActivationFunctionType
OP = mybir.AluOpType
NPBF = ml_dtypes.bfloat16

B, S, H, NH, HD, FF = 2, 2048, 2048, 16, 128, 8192
P = 128
QR = 512            # query rows per core in the FFN phase
HT = H // P         # 16 feature tiles
FT = FF // P        # 64 ffn tiles
NKT = S // P        # 16 key tiles per batch
EPS = 1e-5

# ---------------------------------------------------------------------------
# Workaround for this container's walrus build: it supports only ONE sync
# wait per instruction, but Tile attaches several.  Rewrite the BIR just
# before walrus: an instruction with N>1 waits gets N-1 same-engine NoOps
# inserted before it, each carrying one wait.
# ---------------------------------------------------------------------------


def _split_multiwaits(bir_bytes):
    bir = json.loads(bir_bytes)
    ctr = 0
    for fn in bir.get("functions", []):
        for blk in fn.get("blocks", []):
            new = []
            for inst in blk.get("instructions", []):
                si = inst.get("sync_info")
                waits = (si or {}).get("on_wait") or []
                if len(waits) > 1:
                    for w in waits[:-1]:
                        ctr += 1
                        new.append({
                            "engine": inst["engine"],
                            "ins": [],
                            "outs": [],
                            "name": f"I-mwsplit{ctr}",
                            "opcode": "NoOp",
                            "sync_info": {"on_update": [], "on_wait": [w]},
                            "text_hint": "multiwait_split",
                        })
                    si["on_wait"] = [waits[-1]]
                new.append(inst)
            blk["instructions"] = new
    return json.dumps(bir).encode()


def _patched_compile_bir_kernel(bir_json, tmpdir, neff_name="file.neff", **kw):
    if isinstance(bir_json, str):
        bir_json = bir_json.encode()
    return _orig_compile_bir_kernel(_split_multiwaits(bir_json), tmpdir,
                                    neff_name=neff_name, **kw)


def _install_patch():
    bass2jax.compile_bir_kernel = _patched_compile_bir_kernel


# ---------------------------------------------------------------------------
# Device program
# ---------------------------------------------------------------------------


def _ln_stats_rows(nc, tc, pool, ones_bf, load, n, tag):
    """Feature-dim (partition) LN stats over HT tiles of [P, n] bf16.

    Emits the stats matmuls (max 4 PSUM banks: processes 512-col chunks
    two at a time) and the row math.  Returns (rowm, rowr): [1, n] bf16
    rows of mean and rstd.
    """
    nch = n // 512
    mean4 = pool.tile([P, 512], F32, tag=f"{tag}_m4", bufs=1)
    msq4 = pool.tile([P, 512], F32, tag=f"{tag}_q4", bufs=1)
    with tc.tile_pool(name=f"{tag}_sps", bufs=1, space="PSUM") as sps:
        for h0 in range(0, nch, 2):
            hn = min(2, nch - h0)
            mps = [sps.tile([1, 512], F32, tag=f"{tag}_mps{c}",
                            name=f"{tag}_mps{h0}_{c}") for c in range(hn)]
            qps = [sps.tile([1, 512], F32, tag=f"{tag}_qps{c}",
                            name=f"{tag}_qps{h0}_{c}") for c in range(hn)]
            for i in range(HT):
                xt = load(i)
                xsq = pool.tile([P, 512 * hn], BF16, tag=f"{tag}_sq", bufs=1)
                nc.vector.tensor_mul(
                    xsq[:], xt[:, 512 * h0:512 * (h0 + hn)],
                    xt[:, 512 * h0:512 * (h0 + hn)])
                for c in range(hn):
                    sl = slice(512 * (h0 + c), 512 * (h0 + c + 1))
                    nc.tensor.matmul(mps[c][:], ones_bf[:], xt[:, sl],
                                     start=(i == 0), stop=(i == HT - 1))
                    nc.tensor.matmul(qps[c][:], ones_bf[:],
                                     xsq[:, 512 * c:512 * (c + 1)],
                                     start=(i == 0), stop=(i == HT - 1))
            for c in range(hn):
                r = 32 * (h0 + c)
                nc.scalar.activation(mean4[r:r + 1, :], mps[c][:], AF.Copy,
                                     scale=1.0 / H)
                nc.scalar.activation(msq4[r:r + 1, :], qps[c][:], AF.Copy,
                                     scale=1.0 / H)
    var = pool.tile([P, 512], F32, tag=f"{tag}_var", bufs=1)
    rstd = pool.tile([P, 512], F32, tag=f"{tag}_rstd", bufs=1)
    nc.vector.tensor_mul(var[:], mean4[:], mean4[:])
    nc.vector.tensor_sub(var[:], msq4[:], var[:])
    nc.vector.tensor_scalar_add(var[:], var[:], EPS)
    nc.vector.reciprocal(var[:], var[:])
    nc.scalar.activation(rstd[:], var[:], AF.Sqrt)
    rowm = pool.tile([1, n], BF16, tag=f"{tag}_rowm", bufs=1)
    rowr = pool.tile([1, n], BF16, tag=f"{tag}_rowr", bufs=1)
    for r in range(nch):
        sl = slice(512 * r, 512 * (r + 1))
        nc.scalar.activation(rowm[0:1, sl], mean4[32 * r:32 * r + 1, :],
                             AF.Copy)
        nc.scalar.activation(rowr[0:1, sl], rstd[32 * r:32 * r + 1, :],
                             AF.Copy)
    return rowm, rowr


def _ln_bcast(nc, tc, pool, ones_row_bf, rowm, rowr, n, tag):
    """Broadcast [1, n] mean/rstd rows to [P, n] bf16 tiles via K=1 MMs."""
    nch = n // 512
    bmean = pool.tile([P, n], BF16, tag=f"{tag}_bm", bufs=1)
    brstd = pool.tile([P, n], BF16, tag=f"{tag}_br", bufs=1)
    with tc.tile_pool(name=f"{tag}_bps", bufs=2, space="PSUM") as bps:
        for r in range(nch):
            sl = slice(512 * r, 512 * (r + 1))
            mp = bps.tile([P, 512], F32, tag=f"{tag}_bmp", name=f"{tag}_bmp{r}")
            nc.tensor.matmul(mp[:], ones_row_bf[:], rowm[0:1, sl],
                             start=True, stop=True)
            nc.scalar.activation(bmean[:, sl], mp[:], AF.Copy)
            rp = bps.tile([P, 512], F32, tag=f"{tag}_brp", name=f"{tag}_brp{r}")
            nc.tensor.matmul(rp[:], ones_row_bf[:], rowr[0:1, sl],
                             start=True, stop=True)
            nc.scalar.activation(brstd[:, sl], rp[:], AF.Copy)
    return bmean, brstd


def build_nc(debug_outputs=()):
    _install_patch()
    nc = bass.Bass("TRN2")

    xT_t = nc.dram_tensor("xT_t", (B, P, HT, S), BF16, kind="ExternalInput")
    xq32 = nc.dram_tensor("xq32", (P, HT, QR), F32, kind="ExternalInput")
    wq_h = nc.dram_tensor("wq_h", (P, 2, HT, P), BF16, kind="ExternalInput")
    wk_h = nc.dram_tensor("wk_h", (P, 2, HT, P), BF16, kind="ExternalInput")
    wvT_h = nc.dram_tensor("wvT_h", (P, HT, 2 * P), BF16, kind="ExternalInput")
    wo_h = nc.dram_tensor("wo_h", (HT, P, HT, P), BF16, kind="ExternalInput")
    w1_h = nc.dram_tensor("w1_h", (FT, P, HT, P), BF16, kind="ExternalInput")
    w2_h = nc.dram_tensor("w2_h", (HT, P, FT, P), BF16, kind="ExternalInput")
    mask_h = nc.dram_tensor("mask_h", (P, 4, QR), BF16, kind="ExternalInput")
    ones_d = nc.dram_tensor("ones_d", (P, 1), BF16, kind="ExternalInput")
    ones_r_d = nc.dram_tensor("ones_r_d", (1, P), BF16, kind="ExternalInput")
    bq_d = nc.dram_tensor("bq_d", (P, 2), F32, kind="ExternalInput")
    bk_d = nc.dram_tensor("bk_d", (P, 2), F32, kind="ExternalInput")
    bv_d = nc.dram_tensor("bv_d", (P, 2), F32, kind="ExternalInput")
    bwo_d = nc.dram_tensor("bwo_d", (P, HT), F32, kind="ExternalInput")
    b1_d = nc.dram_tensor("b1_d", (P, FT), F32, kind="ExternalInput")
    b2_d = nc.dram_tensor("b2_d", (P, HT), F32, kind="ExternalInput")
    ln1w_d = nc.dram_tensor("ln1w_d", (P, HT), F32, kind="ExternalInput")
    ln1b_d = nc.dram_tensor("ln1b_d", (P, HT), F32, kind="ExternalInput")
    ln2w_d = nc.dram_tensor("ln2w_d", (P, HT), F32, kind="ExternalInput")
    ln2b_d = nc.dram_tensor("ln2b_d", (P, HT), F32, kind="ExternalInput")
    outT = nc.dram_tensor("outT", (H, QR), F32, kind="ExternalOutput")

    a2a_in = [nc.dram_tensor(f"a2a_in{i}", (8, P, QR), BF16) for i in range(2)]
    a2a_out = [nc.dram_tensor(f"a2a_out{i}", (8, P, QR), BF16)
               for i in range(2)]
    xstat_d = nc.dram_tensor("xstat_d", (P, HT, QR), BF16,
                             kind="ExternalInput")
    stats_in_d = nc.dram_tensor("stats_in_d", (2, 512), F32)
    stats_out_d = nc.dram_tensor("stats_out_d", (8, 2, 512), F32,
                                 addr_space="Shared")

    dbg = {}
    for name, shape, dt in (("q0", (P, 2, S), BF16), ("k0", (P, 2, S), BF16),
                            ("v0", (P, NKT, 2 * P), BF16),
                            ("av0", (P, 8, QR), BF16), ("av1", (P, 8, QR), BF16),
                            ("rx", (P, HT, QR), BF16),
                            ("h", (P, HT, QR), BF16), ("g", (P, HT, QR), BF16)):
        if name in debug_outputs:
            dbg[name] = nc.dram_tensor(f"dbg_{name}", shape, dt,
                                       kind="ExternalOutput")

    with tile.TileContext(nc) as tc:
        cm_const = tc.tile_pool(name="const", bufs=1)
        const = cm_const.__enter__()
        ones_bf = const.tile([P, 1], BF16, tag="ones")
        nc.sync.dma_start(ones_bf[:], ones_d[:])
        ones_row_bf = const.tile([1, P], BF16, tag="ones_row")
        nc.sync.dma_start(ones_row_bf[:], ones_r_d[:])
        mask_sb = const.tile([P, 4, QR], BF16, tag="mask")
        nc.sync.dma_start(mask_sb[:], mask_h[:])
        wq_sb = const.tile([P, 2, HT, P], BF16, tag="wq")
        nc.sync.dma_start(wq_sb[:], wq_h[:])
        wk_sb = const.tile([P, 2, HT, P], BF16, tag="wk")
        nc.sync.dma_start(wk_sb[:], wk_h[:])
        wvT_sb = const.tile([P, HT, 2 * P], BF16, tag="wv")
        nc.sync.dma_start(wvT_sb[:], wvT_h[:])

        def bias_tile(name, dram_t, ntiles):
            t = const.tile([P, ntiles], F32, tag=f"b_{name}")
            nc.sync.dma_start(t[:], dram_t[:])
            return t

        bq_t = bias_tile("bq", bq_d, 2)
        bk_t = bias_tile("bk", bk_d, 2)
        bv_t = bias_tile("bv", bv_d, 2)
        bwo_t = bias_tile("bwo", bwo_d, HT)
        b1_t = bias_tile("b1", b1_d, FT)
        b2_t = bias_tile("b2", b2_d, HT)
        ln1w_t = bias_tile("ln1w", ln1w_d, HT)
        ln1b_t = bias_tile("ln1b", ln1b_d, HT)
        ln2w_t = bias_tile("ln2w", ln2w_d, HT)
        ln2b_t = bias_tile("ln2b", ln2b_d, HT)

        # Persistent per-batch Q/K/V results and attention outputs.
        cm_qkv = tc.tile_pool(name="qkv", bufs=1)
        qkvp = cm_qkv.__enter__()
        q_sb = [qkvp.tile([P, 2, S], BF16, tag=f"q{b}", name=f"q{b}")
                for b in range(B)]
        k_sb = [qkvp.tile([P, 2, S], BF16, tag=f"k{b}", name=f"k{b}")
                for b in range(B)]
        vT_sb = [qkvp.tile([P, NKT, 2 * P], BF16, tag=f"v{b}", name=f"v{b}")
                 for b in range(B)]

        cm_av = tc.tile_pool(name="av", bufs=1)
        avp_ = cm_av.__enter__()
        av_keep = [avp_.tile([P, 8, QR], BF16, tag=f"avk{h}", name=f"avk{h}")
                   for h in range(2)]
        dn_all = [[avp_.tile([P, QR], F32, tag=f"dn{h}{b}",
                              name=f"dnall{h}{b}") for b in range(B)]
                  for h in range(2)]

        # ---- sharded LN1 stats: each core reduces its own 512-col slice,
        # an 8-core AllGather distributes mean/E[x^2] rows for both batches.
        cm_l1 = tc.tile_pool(name="l1rows", bufs=1)
        l1p = cm_l1.__enter__()
        with tc.tile_pool(name="xstat", bufs=1) as xsp, \
             tc.tile_pool(name="stps", bufs=1, space="PSUM") as stps:
            xst = xsp.tile([P, HT, QR], BF16, tag="xst")
            nc.sync.dma_start(xst[:], xstat_d[:])
            mps = stps.tile([1, QR], F32, tag="st_m", name="st_m")
            qps = stps.tile([1, QR], F32, tag="st_q", name="st_q")
            for i in range(HT):
                xsq = xsp.tile([P, QR], BF16, tag="st_sq", bufs=2)
                nc.vector.tensor_mul(xsq[:], xst[:, i, :], xst[:, i, :])
                nc.tensor.matmul(mps[:], ones_bf[:], xst[:, i, :],
                                 start=(i == 0), stop=(i == HT - 1))
                nc.tensor.matmul(qps[:], ones_bf[:], xsq[:],
                                 start=(i == 0), stop=(i == HT - 1))
            stg = xsp.tile([P, QR], F32, tag="stg")
            nc.scalar.activation(stg[0:1, :], mps[:], AF.Copy, scale=1.0 / H)
            nc.scalar.activation(stg[32:33, :], qps[:], AF.Copy, scale=1.0 / H)
            nc.sync.dma_start(stats_in_d[0], stg[0:1, :])
            nc.sync.dma_start(stats_in_d[1], stg[32:33, :])
            nc.gpsimd.collective_compute(
                "AllGather", OP.bypass, replica_groups=[list(range(8))],
                ins=[stats_in_d[:].opt()], outs=[stats_out_d[:].opt()])
        rowmb, rowrb = [], []
        for b in range(B):
            mean4 = l1p.tile([P, 512], F32, tag="ag_m", name=f"ag_m{b}")
            msq4 = l1p.tile([P, 512], F32, tag="ag_q", name=f"ag_q{b}")
            for c in range(4):
                nc.sync.dma_start(mean4[32 * c:32 * c + 1, :],
                                  stats_out_d[4 * b + c, 0:1, :])
                nc.sync.dma_start(msq4[32 * c:32 * c + 1, :],
                                  stats_out_d[4 * b + c, 1:2, :])
            var = l1p.tile([P, 512], F32, tag="ag_v", name=f"ag_v{b}")
            rstd = l1p.tile([P, 512], F32, tag="ag_r", name=f"ag_r{b}")
            nc.vector.tensor_mul(var[:], mean4[:], mean4[:])
            nc.vector.tensor_sub(var[:], msq4[:], var[:])
            nc.vector.tensor_scalar_add(var[:], var[:], EPS)
            nc.vector.reciprocal(var[:], var[:])
            nc.scalar.activation(rstd[:], var[:], AF.Sqrt)
            rowm = l1p.tile([1, S], BF16, tag=f"ag_rowm{b}", name=f"agro{b}")
            rowr = l1p.tile([1, S], BF16, tag=f"ag_rowr{b}", name=f"agrr{b}")
            for c in range(4):
                sl = slice(512 * c, 512 * (c + 1))
                nc.scalar.activation(rowm[0:1, sl], mean4[32 * c:32 * c + 1, :],
                                     AF.Copy)
                nc.scalar.activation(rowr[0:1, sl], rstd[32 * c:32 * c + 1, :],
                                     AF.Copy)
            rowmb.append(rowm)
            rowrb.append(rowr)

        def phase_a_apply_qkv(b):
            cm = tc.tile_pool(name=f"x{b}", bufs=1)
            xp = cm.__enter__()
            x_sb = xp.tile([P, HT, S], BF16, tag=f"x{b}")
            for c4 in range(4):
                for i in range(HT):
                    csl = slice(QR * c4, QR * (c4 + 1))
                    nc.sync.dma_start(x_sb[:, i, csl], xT_t[b, :, i, csl])
            bmean, brstd = _ln_bcast(nc, tc, l1p, ones_row_bf, rowmb[b],
                                     rowrb[b], S, "l1")
            with tc.tile_pool(name=f"prj{b}", bufs=1, space="PSUM") as pps:
                for c4 in range(4):
                    csl = slice(QR * c4, QR * (c4 + 1))
                    for i in range(HT):
                        t1 = xp.tile([P, QR], BF16, tag="t1", bufs=2)
                        nc.vector.tensor_sub(t1[:], x_sb[:, i, csl],
                                             bmean[:, csl])
                        t2 = xp.tile([P, QR], BF16, tag="t2", bufs=2)
                        nc.vector.tensor_mul(t2[:], t1[:], brstd[:, csl])
                        nc.scalar.activation(x_sb[:, i, csl], t2[:],
                                             AF.Identity,
                                             bias=ln1b_t[:, i:i + 1],
                                             scale=ln1w_t[:, i:i + 1])
                    for w_sb, bias_t, dst in ((wq_sb, bq_t, q_sb[b]),
                                              (wk_sb, bk_t, k_sb[b])):
                        for m in range(2):
                            ps = pps.tile([P, QR], F32, tag="qkps", bufs=2,
                                          name=f"qk{b}_{c4}_{id(w_sb)}_{m}")
                            for ht in range(HT):
                                nc.tensor.matmul(
                                    ps[:], w_sb[:, m, ht, :],
                                    x_sb[:, ht, csl],
                                    start=(ht == 0), stop=(ht == HT - 1))
                            nc.vector.tensor_scalar_add(
                                dst[:, m, csl], ps[:], bias_t[:, m:m + 1])
                    for sM in range(4 * c4, 4 * c4 + 4):
                        vp = pps.tile([P, 2 * P], F32, tag="vps", bufs=2,
                                      name=f"v{b}_{sM}")
                        for ht in range(HT):
                            nc.tensor.matmul(vp[:],
                                             x_sb[:, ht, P * sM:P * (sM + 1)],
                                             wvT_sb[:, ht, :],
                                             start=(ht == 0),
                                             stop=(ht == HT - 1))
                        nc.vector.tensor_scalar_add(vT_sb[b][:, sM, :], vp[:],
                                                    0.0)
            cm.__exit__(None, None, None)

        def attn_hb(hd_t, b, ap_, aps):
            for qb in range(4):
                nk = 4 * qb + 4
                pt = ap_.tile([P, NKT, QR], BF16, tag="pt", bufs=1)
                for k2 in range(0, nk, 2):
                    sp2 = aps.tile([P, 2, QR], F32, tag="sp2", bufs=2,
                                   name=f"sp{hd_t}_{b}_{qb}_{k2}")
                    for j in range(2):
                        kt = k2 + j
                        nc.tensor.matmul(
                            sp2[:, j, :],
                            k_sb[b][:, hd_t, P * kt:P * (kt + 1)],
                            q_sb[b][:, hd_t, QR * qb:QR * (qb + 1)],
                            start=True, stop=True)
                    nc.scalar.activation(pt[:, k2:k2 + 2, :], sp2[:], AF.Exp)
                    if k2 >= 4 * qb:
                        j0 = k2 - 4 * qb
                        nc.vector.tensor_mul(pt[:, k2:k2 + 2, :],
                                             pt[:, k2:k2 + 2, :],
                                             mask_sb[:, j0:j0 + 2, :])
                r = 4 * b + qb
                # denominator: fp32 DVE chain over pt tiles, one final
                # partition-sum matmul on a bf16 copy of the total
                acc = ap_.tile([P, QR], F32, tag="dnacc", bufs=2)
                nc.vector.tensor_add(acc[:], pt[:, 0, :], pt[:, 1, :])
                for kt in range(2, nk - 1):
                    nc.vector.tensor_add(acc[:], acc[:], pt[:, kt, :])
                accb = ap_.tile([P, QR], BF16, tag="dnaccb", bufs=2)
                nc.vector.tensor_add(accb[:], acc[:], pt[:, nk - 1, :])
                dnp = aps.tile([1, QR], F32, tag="dn", bufs=1)
                nc.tensor.matmul(dnp[:], ones_bf[:], accb[:],
                                 start=True, stop=True)
                nc.scalar.activation(dn_all[hd_t][b][32 * qb:32 * qb + 1, :],
                                     dnp[:], AF.Copy)
                avp = aps.tile([P, QR], F32, tag="av", bufs=1)
                for kt in range(nk):
                    nc.tensor.matmul(avp[:],
                                     vT_sb[b][:, kt, P * hd_t:P * (hd_t + 1)],
                                     pt[:, kt, :],
                                     start=(kt == 0), stop=(kt == nk - 1))
                nc.scalar.activation(av_keep[hd_t][:, r, :], avp[:], AF.Copy)

        def norm_prep(hd_t, np_):
            rrow = np_.tile([1, 8 * QR], BF16, tag=f"rrow{hd_t}",
                            name=f"rrow{hd_t}", bufs=1)
            for b in range(B):
                rec = np_.tile([P, QR], F32, tag="rec", bufs=1)
                nc.vector.reciprocal(rec[:], dn_all[hd_t][b][:])
                recbf = np_.tile([P, QR], BF16, tag="recbf", bufs=1)
                nc.scalar.activation(recbf[:], rec[:], AF.Copy)
                for qb in range(4):
                    r = 4 * b + qb
                    nc.scalar.activation(rrow[0:1, QR * r:QR * (r + 1)],
                                         recbf[32 * qb:32 * qb + 1, :],
                                         AF.Copy)
            return rrow

        def norm_fire(hd_t, np_, rrow):
            with tc.tile_pool(name=f"nps{hd_t}", bufs=2, space="PSUM") as nps:
                for r in range(8):
                    bp = nps.tile([P, QR], F32, tag="brec", name=f"brc{hd_t}{r}")
                    nc.tensor.matmul(bp[:], ones_row_bf[:],
                                     rrow[0:1, QR * r:QR * (r + 1)],
                                     start=True, stop=True)
                    brec = np_.tile([P, QR], BF16, tag="brecs", bufs=1)
                    nc.scalar.activation(brec[:], bp[:], AF.Copy)
                    st = np_.tile([P, QR], BF16, tag="avst", bufs=1)
                    nc.vector.tensor_mul(st[:], av_keep[hd_t][:, r, :], brec[:])
                    nc.vector.tensor_scalar_add(st[:], st[:],
                                                bv_t[:, hd_t:hd_t + 1])
                    nc.sync.dma_start(a2a_in[hd_t][r], st[:])
            nc.gpsimd.collective_compute(
                "AllToAll", OP.bypass, replica_groups=[list(range(8))],
                ins=[a2a_in[hd_t][:].opt()], outs=[a2a_out[hd_t][:].opt()])

        # ---- schedule ----
        phase_a_apply_qkv(0)
        phase_a_apply_qkv(1)
        cm_l1.__exit__(None, None, None)
        cm_attn = tc.tile_pool(name="attn", bufs=1)
        atp = cm_attn.__enter__()
        cm_aps = tc.tile_pool(name="attnps", bufs=2, space="PSUM")
        aps = cm_aps.__enter__()
        attn_hb(0, 0, atp, aps)
        attn_hb(0, 1, atp, aps)
        cm_norm = tc.tile_pool(name="norm", bufs=1)
        np_ = cm_norm.__enter__()
        rrow0 = norm_prep(0, np_)
        attn_hb(1, 0, atp, aps)
        norm_fire(0, np_, rrow0)
        attn_hb(1, 1, atp, aps)
        rrow1 = norm_prep(1, np_)
        norm_fire(1, np_, rrow1)

        if "q0" in dbg:
            nc.sync.dma_start(dbg["q0"][:], q_sb[0][:])
        if "k0" in dbg:
            nc.sync.dma_start(dbg["k0"][:], k_sb[0][:])
        if "v0" in dbg:
            nc.sync.dma_start(dbg["v0"][:], vT_sb[0][:])
        for hh in range(2):
            if f"av{hh}" in dbg:
                nc.sync.dma_start(dbg[f"av{hh}"][:], av_keep[hh][:])

        cm_norm.__exit__(None, None, None)
        cm_aps.__exit__(None, None, None)
        cm_attn.__exit__(None, None, None)
        cm_av.__exit__(None, None, None)
        cm_qkv.__exit__(None, None, None)

        # ---- phase C: WO + residual, LN2, FFN ----
        cm_c = tc.tile_pool(name="cres", bufs=1)
        cp = cm_c.__enter__()
        rx = cp.tile([P, HT, QR], BF16, tag="rx")
        nc.sync.dma_start(rx[:, 0:8, :], a2a_out[0].rearrange("j p q -> p j q"))
        nc.sync.dma_start(rx[:, 8:16, :],
                          a2a_out[1].rearrange("j p q -> p j q"))
        if "rx" in dbg:
            nc.sync.dma_start(dbg["rx"][:], rx[:])
        h_bf = cp.tile([P, HT, QR], BF16, tag="hbf")
        woA = cp.tile([P, HT, QR], F32, tag="woA")
        with tc.tile_pool(name="wo", bufs=2) as wop, \
             tc.tile_pool(name="wops", bufs=2, space="PSUM") as wops:
            for dM in range(HT):
                wot = wop.tile([P, 8, P], BF16, tag="wotA")
                nc.sync.dma_start(wot[:], wo_h[dM][:, 0:8, :])
                ps = wops.tile([P, QR], F32, tag="wops")
                for kt in range(8):
                    nc.tensor.matmul(ps[:], wot[:, kt, :], rx[:, kt, :],
                                     start=(kt == 0), stop=(kt == 7))
                nc.scalar.activation(woA[:, dM, :], ps[:], AF.Copy)
            for dM in range(HT):
                wot = wop.tile([P, 8, P], BF16, tag="wotB")
                nc.sync.dma_start(wot[:], wo_h[dM][:, 8:16, :])
                xq_t = wop.tile([P, QR], F32, tag="xqt")
                nc.sync.dma_start(xq_t[:], xq32[:, dM, :])
                ps = wops.tile([P, QR], F32, tag="wops")
                for kt in range(8):
                    nc.tensor.matmul(ps[:], wot[:, kt, :], rx[:, 8 + kt, :],
                                     start=(kt == 0), stop=(kt == 7))
                tsum = wop.tile([P, QR], F32, tag="tsum")
                nc.vector.tensor_add(tsum[:], ps[:], woA[:, dM, :])
                nc.vector.scalar_tensor_tensor(
                    h_bf[:, dM, :], tsum[:], bwo_t[:, dM:dM + 1],
                    xq_t[:], op0=OP.add, op1=OP.add)
        if "h" in dbg:
            nc.sync.dma_start(dbg["h"][:], h_bf[:])

        g_sb = cp.tile([P, HT, QR], BF16, tag="g")
        with tc.tile_pool(name="ln2", bufs=1) as l2p:
            rowm2, rowr2 = _ln_stats_rows(nc, tc, l2p, ones_bf,
                                          lambda i: h_bf[:, i, :], QR, "l2")
            bmean2, brstd2 = _ln_bcast(nc, tc, l2p, ones_row_bf, rowm2, rowr2,
                                       QR, "l2")
            for i in range(HT):
                t1 = l2p.tile([P, QR], BF16, tag="t1", bufs=2)
                nc.vector.tensor_sub(t1[:], h_bf[:, i, :], bmean2[:])
                t2 = l2p.tile([P, QR], BF16, tag="t2", bufs=2)
                nc.vector.tensor_mul(t2[:], t1[:], brstd2[:])
                nc.scalar.activation(g_sb[:, i, :], t2[:], AF.Identity,
                                     bias=ln2b_t[:, i:i + 1],
                                     scale=ln2w_t[:, i:i + 1])
        if "g" in dbg:
            nc.sync.dma_start(dbg["g"][:], g_sb[:])

        f_sb = cp.tile([P, FT, QR], BF16, tag="f")
        with tc.tile_pool(name="ffn1", bufs=3) as f1p, \
             tc.tile_pool(name="f1ps", bufs=2, space="PSUM") as f1ps:
            for fM in range(FT):
                w1t = f1p.tile([P, HT, P], BF16, tag="w1t")
                nc.sync.dma_start(w1t[:], w1_h[fM])
                ps = f1ps.tile([P, QR], F32, tag="f1ps")
                for ht in range(HT):
                    nc.tensor.matmul(ps[:], w1t[:, ht, :], g_sb[:, ht, :],
                                     start=(ht == 0), stop=(ht == HT - 1))
                nc.scalar.activation(f_sb[:, fM, :], ps[:], AF.Gelu,
                                     bias=b1_t[:, fM:fM + 1])

        with tc.tile_pool(name="ffn2", bufs=2) as f2p, \
             tc.tile_pool(name="f2ps", bufs=2, space="PSUM") as f2ps:
            for dM in range(HT):
                ps = f2ps.tile([P, QR], F32, tag="f2ps")
                for q4 in range(4):
                    w2t = f2p.tile([P, HT, P], BF16, tag="w2t")
                    nc.sync.dma_start(w2t[:], w2_h[dM][:, HT * q4:HT * (q4 + 1), :])
                    for ft in range(HT):
                        kk = HT * q4 + ft
                        nc.tensor.matmul(ps[:], w2t[:, ft, :], f_sb[:, kk, :],
                                         start=(kk == 0), stop=(kk == FT - 1))
                ost = f2p.tile([P, QR], F32, tag="ost")
                nc.vector.scalar_tensor_tensor(
                    ost[:], ps[:], b2_t[:, dM:dM + 1], h_bf[:, dM, :],
                    op0=OP.add, op1=OP.add)
                nc.sync.dma_start(outT[P * dM:P * (dM + 1), :], ost[:])
        cm_c.__exit__(None, None, None)
        cm_const.__exit__(None, None, None)

    return nc


# ---------------------------------------------------------------------------
# Host side
# ---------------------------------------------------------------------------

_CACHE = {}


def _get_nc(debug_outputs=()):
    key = tuple(sorted(debug_outputs))
    if key not in _CACHE:
        _CACHE[key] = build_nc(debug_outputs)
    return _CACHE[key]


def tile_kxm(wT):
    # [K, M] -> [mM, p(K), kt, m2] so each [128, kt*128] lhsT load is
    # contiguous per partition.
    K_, M_ = wT.shape
    return np.ascontiguousarray(
        wT.reshape(K_ // P, P, M_ // P, P).transpose(2, 1, 0, 3))


def pcol(v):
    # [n*P] -> [P, n] bias tile (column m holds elements m*P..m*P+P-1)
    return np.ascontiguousarray(
        np.asarray(v, np.float32).reshape(-1, P).T)


def make_in_maps(inputs):
    x = np.asarray(inputs["x"], np.float32)
    scale = np.float32(1.0 / np.sqrt(HD))
    wqkv = np.asarray(inputs["wqkv_w"], np.float32)
    wqkv_b = np.asarray(inputs["wqkv_b"], np.float32)

    xT_t = np.ascontiguousarray(
        x.transpose(0, 2, 1).reshape(B, HT, P, S).transpose(0, 2, 1, 3)
    ).astype(NPBF)

    mask = np.zeros((P, 4, QR), np.float32)
    r_ = np.arange(P)[:, None]
    c_ = np.arange(QR)[None, :]
    for j in range(4):
        mask[:, j, :] = (P * j + r_ <= c_).astype(np.float32)

    shared = {
        "xT_t": xT_t,
        "mask_h": mask.astype(NPBF),
        "ones_d": np.ones((P, 1), NPBF),
        "ones_r_d": np.ones((1, P), NPBF),
        "wo_h": tile_kxm(np.asarray(inputs["wo_w"], np.float32).T).astype(NPBF),
        "w1_h": tile_kxm(np.asarray(inputs["w1"], np.float32).T).astype(NPBF),
        "w2_h": tile_kxm(np.asarray(inputs["w2"], np.float32).T).astype(NPBF),
        "bwo_d": pcol(inputs["wo_b"]),
        "b1_d": pcol(inputs["b1"]),
        "b2_d": pcol(inputs["b2"]),
        "ln1w_d": pcol(inputs["ln1_w"]),
        "ln1b_d": pcol(inputs["ln1_b"]),
        "ln2w_d": pcol(inputs["ln2_w"]),
        "ln2b_d": pcol(inputs["ln2_b"]),
    }
    in_maps = []
    for core in range(8):
        rows = np.r_[core * P:(core + 1) * P,
                     (core + 8) * P:(core + 9) * P]
        wq_s = wqkv[:H][rows] * scale
        wk_s = wqkv[H:2 * H][rows]
        wv_s = wqkv[2 * H:][rows]
        b_out, c_out = divmod(core, 4)
        xq = x[b_out, QR * c_out:QR * (c_out + 1), :].T  # [H, QR]
        xq_t = np.ascontiguousarray(xq.reshape(HT, P, QR).transpose(1, 0, 2))
        in_maps.append(dict(
            shared,
            xq32=xq_t,
            xstat_d=xq_t.astype(NPBF),
            wq_h=np.ascontiguousarray(
                tile_kxm(wq_s.T).transpose(1, 0, 2, 3)).astype(NPBF),
            wk_h=np.ascontiguousarray(
                tile_kxm(wk_s.T).transpose(1, 0, 2, 3)).astype(NPBF),
            wvT_h=np.ascontiguousarray(
                wv_s.T.reshape(HT, P, 2 * P).transpose(1, 0, 2)).astype(NPBF),
            bq_d=np.ascontiguousarray(
                (wqkv_b[:H][rows] * scale).reshape(2, P).T),
            bk_d=np.ascontiguousarray(wqkv_b[H:2 * H][rows].reshape(2, P).T),
            bv_d=np.ascontiguousarray(wqkv_b[2 * H:][rows].reshape(2, P).T),
        ))
    return in_maps


def run_cores(inputs, debug_outputs=(), **run_kw):
    nc = _get_nc(debug_outputs)
    in_maps = make_in_maps(inputs)
    return nc, run_bass_kernel_spmd(nc, in_maps, core_ids=list(range(8)),
                                    **run_kw)


def kernel(**inputs):
    _, res = run_cores(inputs)
    out = np.empty((B, S, H), np.float32)
    for core in range(8):
        b, c = divmod(core, 4)
        out[b, QR * c:QR * (c + 1), :] = res.results[core]["outT"].T
    return out


# revision 4
# speedup vs baseline: 1.2415x; 1.0096x over previous
"""Trainium2 Bass kernel v2 for the dense transformer decoder block.

Problem: B=2, S=2048, H=2048, NH=16 (head_dim=128), FFN=8192, fp32 in/out.

Sharding: head-parallel attention + query-parallel FFN, stitched by two
1MB AllToAlls.  Core g computes LN1 + Q/K/V projections for heads
{g, g+8} over BOTH batches (no redundant K/V work), runs causally-tiled
attention for those heads (upper-triangle tiles skipped; only
diagonal-crossing tiles pay a 0/1-mask multiply), then an 8-core
AllToAll redistributes the attention output so core g holds ALL 2048
features for its 512-query block (b=g//4, rows 512*(g%4)...).  WO, LN2
and the FFN then run on that block exactly once per core.

All matmuls run in bf16 (fp32 PSUM accumulation): same PE stream rate
as f32r but half the DMA/SBUF and 2x faster weight loads; rel-err
budget (2e-2) has plenty of headroom.  LayerNorm/softmax partition-dim
statistics use ones-vector matmuls; softmax denominators for a head are
batched into a single [8,512] vector reciprocal.
"""

import json

import numpy as np
import ml_dtypes

import concourse.bass as bass
import concourse.bass2jax as bass2jax
import concourse.mybir as mybir
import concourse.tile as tile
from concourse.bass_utils import compile_bir_kernel as _orig_compile_bir_kernel
from concourse.bass_utils import run_bass_kernel_spmd

F32 = mybir.dt.float32
BF16 = mybir.dt.bfloat16
AF = mybir.ActivationFunctionType
OP = mybir.AluOpType
NPBF = ml_dtypes.bfloat16

B, S, H, NH, HD, FF = 2, 2048, 2048, 16, 128, 8192
P = 128
QR = 512            # query rows per core in the FFN phase
HT = H // P         # 16 feature tiles
FT = FF // P        # 64 ffn tiles
NKT = S // P        # 16 key tiles per batch
EPS = 1e-5

# ---------------------------------------------------------------------------
# Workaround for this container's walrus build: it supports only ONE sync
# wait per instruction, but Tile attaches several.  Rewrite the BIR just
# before walrus: an instruction with N>1 waits gets N-1 same-engine NoOps
# inserted before it, each carrying one wait.
# ---------------------------------------------------------------------------


def _split_multiwaits(bir_bytes):
    bir = json.loads(bir_bytes)
    ctr = 0
    for fn in bir.get("functions", []):
        for blk in fn.get("blocks", []):
            new = []
            for inst in blk.get("instructions", []):
                si = inst.get("sync_info")
                waits = (si or {}).get("on_wait") or []
                if len(waits) > 1:
                    for w in waits[:-1]:
                        ctr += 1
                        new.append({
                            "engine": inst["engine"],
                            "ins": [],
                            "outs": [],
                            "name": f"I-mwsplit{ctr}",
                            "opcode": "NoOp",
                            "sync_info": {"on_update": [], "on_wait": [w]},
                            "text_hint": "multiwait_split",
                        })
                    si["on_wait"] = [waits[-1]]
                new.append(inst)
            blk["instructions"] = new
    return json.dumps(bir).encode()


def _patched_compile_bir_kernel(bir_json, tmpdir, neff_name="file.neff", **kw):
    if isinstance(bir_json, str):
        bir_json = bir_json.encode()
    return _orig_compile_bir_kernel(_split_multiwaits(bir_json), tmpdir,
                                    neff_name=neff_name, **kw)


def _install_patch():
    bass2jax.compile_bir_kernel = _patched_compile_bir_kernel


# ---------------------------------------------------------------------------
# Device program
# ---------------------------------------------------------------------------


def _ln_stats_rows(nc, tc, pool, ones_bf, load, n, tag):
    """Feature-dim (partition) LN stats over HT tiles of [P, n] bf16.

    Emits the stats matmuls (max 4 PSUM banks: processes 512-col chunks
    two at a time) and the row math.  Returns (rowm, rowr): [1, n] bf16
    rows of mean and rstd.
    """
    nch = n // 512
    mean4 = pool.tile([P, 512], F32, tag=f"{tag}_m4", bufs=1)
    msq4 = pool.tile([P, 512], F32, tag=f"{tag}_q4", bufs=1)
    with tc.tile_pool(name=f"{tag}_sps", bufs=1, space="PSUM") as sps:
        for h0 in range(0, nch, 2):
            hn = min(2, nch - h0)
            mps = [sps.tile([1, 512], F32, tag=f"{tag}_mps{c}",
                            name=f"{tag}_mps{h0}_{c}") for c in range(hn)]
            qps = [sps.tile([1, 512], F32, tag=f"{tag}_qps{c}",
                            name=f"{tag}_qps{h0}_{c}") for c in range(hn)]
            for i in range(HT):
                xt = load(i)
                xsq = pool.tile([P, 512 * hn], BF16, tag=f"{tag}_sq", bufs=1)
                nc.vector.tensor_mul(
                    xsq[:], xt[:, 512 * h0:512 * (h0 + hn)],
                    xt[:, 512 * h0:512 * (h0 + hn)])
                for c in range(hn):
                    sl = slice(512 * (h0 + c), 512 * (h0 + c + 1))
                    nc.tensor.matmul(mps[c][:], ones_bf[:], xt[:, sl],
                                     start=(i == 0), stop=(i == HT - 1))
                    nc.tensor.matmul(qps[c][:], ones_bf[:],
                                     xsq[:, 512 * c:512 * (c + 1)],
                                     start=(i == 0), stop=(i == HT - 1))
            for c in range(hn):
                r = 32 * (h0 + c)
                nc.scalar.activation(mean4[r:r + 1, :], mps[c][:], AF.Copy,
                                     scale=1.0 / H)
                nc.scalar.activation(msq4[r:r + 1, :], qps[c][:], AF.Copy,
                                     scale=1.0 / H)
    var = pool.tile([P, 512], F32, tag=f"{tag}_var", bufs=1)
    rstd = pool.tile([P, 512], F32, tag=f"{tag}_rstd", bufs=1)
    nc.vector.tensor_mul(var[:], mean4[:], mean4[:])
    nc.vector.tensor_sub(var[:], msq4[:], var[:])
    nc.vector.tensor_scalar_add(var[:], var[:], EPS)
    nc.vector.reciprocal(var[:], var[:])
    nc.scalar.activation(rstd[:], var[:], AF.Sqrt)
    rowm = pool.tile([1, n], BF16, tag=f"{tag}_rowm", bufs=1)
    rowr = pool.tile([1, n], BF16, tag=f"{tag}_rowr", bufs=1)
    for r in range(nch):
        sl = slice(512 * r, 512 * (r + 1))
        nc.scalar.activation(rowm[0:1, sl], mean4[32 * r:32 * r + 1, :],
                             AF.Copy)
        nc.scalar.activation(rowr[0:1, sl], rstd[32 * r:32 * r + 1, :],
                             AF.Copy)
    return rowm, rowr


def _ln_bcast(nc, tc, pool, ones_row_bf, rowm, rowr, n, tag):
    """Broadcast [1, n] mean/rstd rows to [P, n] bf16 tiles via K=1 MMs."""
    nch = n // 512
    bmean = pool.tile([P, n], BF16, tag=f"{tag}_bm", bufs=1)
    brstd = pool.tile([P, n], BF16, tag=f"{tag}_br", bufs=1)
    with tc.tile_pool(name=f"{tag}_bps", bufs=2, space="PSUM") as bps:
        for r in range(nch):
            sl = slice(512 * r, 512 * (r + 1))
            mp = bps.tile([P, 512], F32, tag=f"{tag}_bmp", name=f"{tag}_bmp{r}")
            nc.tensor.matmul(mp[:], ones_row_bf[:], rowm[0:1, sl],
                             start=True, stop=True)
            nc.scalar.activation(bmean[:, sl], mp[:], AF.Copy)
            rp = bps.tile([P, 512], F32, tag=f"{tag}_brp", name=f"{tag}_brp{r}")
            nc.tensor.matmul(rp[:], ones_row_bf[:], rowr[0:1, sl],
                             start=True, stop=True)
            nc.scalar.activation(brstd[:, sl], rp[:], AF.Copy)
    return bmean, brstd


def build_nc(debug_outputs=()):
    _install_patch()
    nc = bass.Bass("TRN2")

    xT_t = nc.dram_tensor("xT_t", (B, P, HT, S), BF16, kind="ExternalInput")
    xq32 = nc.dram_tensor("xq32", (P, HT, QR), F32, kind="ExternalInput")
    wq_h = nc.dram_tensor("wq_h", (P, 2, HT, P), BF16, kind="ExternalInput")
    wk_h = nc.dram_tensor("wk_h", (P, 2, HT, P), BF16, kind="ExternalInput")
    wvT_h = nc.dram_tensor("wvT_h", (P, HT, 2 * P), BF16, kind="ExternalInput")
    wo_h = nc.dram_tensor("wo_h", (HT, P, HT, P), BF16, kind="ExternalInput")
    w1_h = nc.dram_tensor("w1_h", (FT, P, HT, P), BF16, kind="ExternalInput")
    w2_h = nc.dram_tensor("w2_h", (HT, P, FT, P), BF16, kind="ExternalInput")
    mask_h = nc.dram_tensor("mask_h", (P, 4, QR), BF16, kind="ExternalInput")
    ones_d = nc.dram_tensor("ones_d", (P, 1), BF16, kind="ExternalInput")
    ones_r_d = nc.dram_tensor("ones_r_d", (1, P), BF16, kind="ExternalInput")
    bq_d = nc.dram_tensor("bq_d", (P, 2), F32, kind="ExternalInput")
    bk_d = nc.dram_tensor("bk_d", (P, 2), F32, kind="ExternalInput")
    bv_d = nc.dram_tensor("bv_d", (P, 2), F32, kind="ExternalInput")
    bwo_d = nc.dram_tensor("bwo_d", (P, HT), F32, kind="ExternalInput")
    b1_d = nc.dram_tensor("b1_d", (P, FT), F32, kind="ExternalInput")
    b2_d = nc.dram_tensor("b2_d", (P, HT), F32, kind="ExternalInput")
    ln1w_d = nc.dram_tensor("ln1w_d", (P, HT), F32, kind="ExternalInput")
    ln1b_d = nc.dram_tensor("ln1b_d", (P, HT), F32, kind="ExternalInput")
    ln2w_d = nc.dram_tensor("ln2w_d", (P, HT), F32, kind="ExternalInput")
    ln2b_d = nc.dram_tensor("ln2b_d", (P, HT), F32, kind="ExternalInput")
    outT = nc.dram_tensor("outT", (H, QR), F32, kind="ExternalOutput")

    a2a_in = [nc.dram_tensor(f"a2a_in{i}", (8, P, QR), BF16) for i in range(2)]
    a2a_out = [nc.dram_tensor(f"a2a_out{i}", (8, P, QR), BF16)
               for i in range(2)]
    xstat_d = nc.dram_tensor("xstat_d", (P, HT, QR), BF16,
                             kind="ExternalInput")
    stats_in_d = nc.dram_tensor("stats_in_d", (2, 512), F32)
    stats_out_d = nc.dram_tensor("stats_out_d", (8, 2, 512), F32,
                                 addr_space="Shared")

    dbg = {}
    for name, shape, dt in (("q0", (P, 2, S), BF16), ("k0", (P, 2, S), BF16),
                            ("v0", (P, NKT, 2 * P), BF16),
                            ("av0", (P, 8, QR), BF16), ("av1", (P, 8, QR), BF16),
                            ("rx", (P, HT, QR), BF16),
                            ("h", (P, HT, QR), BF16), ("g", (P, HT, QR), BF16)):
        if name in debug_outputs:
            dbg[name] = nc.dram_tensor(f"dbg_{name}", shape, dt,
                                       kind="ExternalOutput")

    with tile.TileContext(nc) as tc:
        cm_const = tc.tile_pool(name="const", bufs=1)
        const = cm_const.__enter__()
        ones_bf = const.tile([P, 1], BF16, tag="ones")
        nc.sync.dma_start(ones_bf[:], ones_d[:])
        ones_row_bf = const.tile([1, P], BF16, tag="ones_row")
        nc.sync.dma_start(ones_row_bf[:], ones_r_d[:])
        wq_sb = const.tile([P, 2, HT, P], BF16, tag="wq")
        wk_sb = const.tile([P, 2, HT, P], BF16, tag="wk")
        wvT_sb = const.tile([P, HT, 2 * P], BF16, tag="wv")

        def bias_tile(name, ntiles):
            return const.tile([P, ntiles], F32, tag=f"b_{name}",
                              name=f"b_{name}")

        bq_t = bias_tile("bq", 2)
        bk_t = bias_tile("bk", 2)
        bv_t = bias_tile("bv", 2)
        bwo_t = bias_tile("bwo", HT)
        b1_t = bias_tile("b1", FT)
        b2_t = bias_tile("b2", HT)
        ln1w_t = bias_tile("ln1w", HT)
        ln1b_t = bias_tile("ln1b", HT)
        ln2w_t = bias_tile("ln2w", HT)
        ln2b_t = bias_tile("ln2b", HT)

        def load_consts():
            nc.sync.dma_start(wq_sb[:], wq_h[:])
            nc.sync.dma_start(wk_sb[:], wk_h[:])
            nc.sync.dma_start(wvT_sb[:], wvT_h[:])
            for t, dr in ((bq_t, bq_d), (bk_t, bk_d), (bv_t, bv_d),
                          (bwo_t, bwo_d), (b1_t, b1_d), (b2_t, b2_d),
                          (ln1w_t, ln1w_d), (ln1b_t, ln1b_d),
                          (ln2w_t, ln2w_d), (ln2b_t, ln2b_d)):
                nc.sync.dma_start(t[:], dr[:])

        cm_rx = tc.tile_pool(name="rxp", bufs=1)
        rxp_ = cm_rx.__enter__()
        rx = rxp_.tile([P, HT, QR], BF16, tag="rx")

        # Persistent per-batch Q/K/V results and attention outputs.
        cm_qkv = tc.tile_pool(name="qkv", bufs=1)
        qkvp = cm_qkv.__enter__()
        q_sb = [qkvp.tile([P, 2, S], BF16, tag=f"q{b}", name=f"q{b}")
                for b in range(B)]
        k_sb = [qkvp.tile([P, 2, S], BF16, tag=f"k{b}", name=f"k{b}")
                for b in range(B)]
        vT_sb = [qkvp.tile([P, NKT, 2 * P], BF16, tag=f"v{b}", name=f"v{b}")
                 for b in range(B)]

        cm_av = tc.tile_pool(name="av", bufs=1)
        avp_ = cm_av.__enter__()
        av_keep = [avp_.tile([P, 8, QR], BF16, tag=f"avk{h}", name=f"avk{h}")
                   for h in range(2)]
        dn_all = [[avp_.tile([P, QR], F32, tag=f"dn{h}{b}",
                              name=f"dnall{h}{b}") for b in range(B)]
                  for h in range(2)]

        # ---- sharded LN1 stats: each core reduces its own 512-col slice,
        # an 8-core AllGather distributes mean/E[x^2] rows for both batches.
        cm_l1 = tc.tile_pool(name="l1rows", bufs=1)
        l1p = cm_l1.__enter__()
        with tc.tile_pool(name="xstat", bufs=1) as xsp, \
             tc.tile_pool(name="stps", bufs=1, space="PSUM") as stps:
            xst = xsp.tile([P, HT, QR], BF16, tag="xst")
            for i in range(HT):
                nc.sync.dma_start(xst[:, i, :], xstat_d[:, i, :])
            mps = stps.tile([1, QR], F32, tag="st_m", name="st_m")
            qps = stps.tile([1, QR], F32, tag="st_q", name="st_q")
            for i in range(HT):
                xsq = xsp.tile([P, QR], BF16, tag="st_sq", bufs=2)
                nc.vector.tensor_mul(xsq[:], xst[:, i, :], xst[:, i, :])
                nc.tensor.matmul(mps[:], ones_bf[:], xst[:, i, :],
                                 start=(i == 0), stop=(i == HT - 1))
                nc.tensor.matmul(qps[:], ones_bf[:], xsq[:],
                                 start=(i == 0), stop=(i == HT - 1))
            stg = xsp.tile([P, QR], F32, tag="stg")
            nc.scalar.activation(stg[0:1, :], mps[:], AF.Copy, scale=1.0 / H)
            nc.scalar.activation(stg[32:33, :], qps[:], AF.Copy, scale=1.0 / H)
            nc.sync.dma_start(stats_in_d[0], stg[0:1, :])
            nc.sync.dma_start(stats_in_d[1], stg[32:33, :])
            nc.gpsimd.collective_compute(
                "AllGather", OP.bypass, replica_groups=[list(range(8))],
                ins=[stats_in_d[:].opt()], outs=[stats_out_d[:].opt()])
        load_consts()
        rowmb, rowrb = [], []
        for b in range(B):
            mean4 = l1p.tile([P, 512], F32, tag="ag_m", name=f"ag_m{b}")
            msq4 = l1p.tile([P, 512], F32, tag="ag_q", name=f"ag_q{b}")
            for c in range(4):
                nc.sync.dma_start(mean4[32 * c:32 * c + 1, :],
                                  stats_out_d[4 * b + c, 0:1, :])
                nc.sync.dma_start(msq4[32 * c:32 * c + 1, :],
                                  stats_out_d[4 * b + c, 1:2, :])
            rowm = l1p.tile([1, S], BF16, tag=f"ag_rowm{b}", name=f"agro{b}")
            rowr = l1p.tile([1, S], BF16, tag=f"ag_rowr{b}", name=f"agrr{b}")
            for c in range(4):
                sl = slice(512 * c, 512 * (c + 1))
                nc.scalar.activation(rowm[0:1, sl], mean4[32 * c:32 * c + 1, :],
                                     AF.Copy)
            nc.vector.tensor_mul(mean4[:], mean4[:], mean4[:])
            nc.vector.tensor_sub(msq4[:], msq4[:], mean4[:])
            nc.vector.tensor_scalar_add(msq4[:], msq4[:], EPS)
            nc.vector.reciprocal(msq4[:], msq4[:])
            nc.scalar.activation(mean4[:], msq4[:], AF.Sqrt)
            for c in range(4):
                sl = slice(512 * c, 512 * (c + 1))
                nc.scalar.activation(rowr[0:1, sl], mean4[32 * c:32 * c + 1, :],
                                     AF.Copy)
            rowmb.append(rowm)
            rowrb.append(rowr)

        def phase_a_apply_qkv(b):
            cm = tc.tile_pool(name=f"x{b}", bufs=1)
            xp = cm.__enter__()
            x_sb = xp.tile([P, HT, S], BF16, tag=f"x{b}")
            for c4 in range(4):
                for i in range(HT):
                    csl = slice(QR * c4, QR * (c4 + 1))
                    nc.sync.dma_start(x_sb[:, i, csl], xT_t[b, :, i, csl])
            bmean, brstd = _ln_bcast(nc, tc, l1p, ones_row_bf, rowmb[b],
                                     rowrb[b], S, "l1")
            with tc.tile_pool(name=f"prj{b}", bufs=1, space="PSUM") as pps:
                for c4 in range(4):
                    csl = slice(QR * c4, QR * (c4 + 1))
                    for i in range(HT):
                        t1 = xp.tile([P, QR], BF16, tag="t1", bufs=1)
                        nc.vector.tensor_sub(t1[:], x_sb[:, i, csl],
                                             bmean[:, csl])
                        t2 = xp.tile([P, QR], BF16, tag="t2", bufs=1)
                        nc.vector.tensor_mul(t2[:], t1[:], brstd[:, csl])
                        nc.scalar.activation(x_sb[:, i, csl], t2[:],
                                             AF.Identity,
                                             bias=ln1b_t[:, i:i + 1],
                                             scale=ln1w_t[:, i:i + 1])
                    for w_sb, bias_t, dst in ((wq_sb, bq_t, q_sb[b]),
                                              (wk_sb, bk_t, k_sb[b])):
                        for m in range(2):
                            ps = pps.tile([P, QR], F32, tag="qkps", bufs=2,
                                          name=f"qk{b}_{c4}_{id(w_sb)}_{m}")
                            for ht in range(HT):
                                nc.tensor.matmul(
                                    ps[:], w_sb[:, m, ht, :],
                                    x_sb[:, ht, csl],
                                    start=(ht == 0), stop=(ht == HT - 1))
                            nc.vector.tensor_scalar_add(
                                dst[:, m, csl], ps[:], bias_t[:, m:m + 1])
                    for sM in range(4 * c4, 4 * c4 + 4):
                        vp = pps.tile([P, 2 * P], F32, tag="vps", bufs=2,
                                      name=f"v{b}_{sM}")
                        for ht in range(HT):
                            nc.tensor.matmul(vp[:],
                                             x_sb[:, ht, P * sM:P * (sM + 1)],
                                             wvT_sb[:, ht, :],
                                             start=(ht == 0),
                                             stop=(ht == HT - 1))
                        nc.vector.tensor_scalar_add(vT_sb[b][:, sM, :], vp[:],
                                                    0.0)
            cm.__exit__(None, None, None)

        def attn_hb(hd_t, b, ap_, aps):
            for qb in range(4):
                nk = 4 * qb + 4
                pt = ap_.tile([P, NKT, QR], BF16, tag="pt", bufs=2)
                avp = aps.tile([P, QR], F32, tag="av", bufs=1,
                               name=f"avp{hd_t}_{b}_{qb}")
                acc = ap_.tile([P, QR], F32, tag="dnacc", bufs=2)
                accb = ap_.tile([P, QR], BF16, tag="dnaccb", bufs=2)

                def av_pair(k2):
                    for j in range(2):
                        kt = k2 + j
                        nc.tensor.matmul(
                            avp[:],
                            vT_sb[b][:, kt, P * hd_t:P * (hd_t + 1)],
                            pt[:, kt, :],
                            start=(kt == 0), stop=(kt == nk - 1),
                            skip_group_check=True)

                for k2 in range(0, nk, 2):
                    sp2 = aps.tile([P, 2, QR], F32, tag="sp2", bufs=2,
                                   name=f"sp{hd_t}_{b}_{qb}_{k2}")
                    for j in range(2):
                        kt = k2 + j
                        nc.tensor.matmul(
                            sp2[:, j, :],
                            k_sb[b][:, hd_t, P * kt:P * (kt + 1)],
                            q_sb[b][:, hd_t, QR * qb:QR * (qb + 1)],
                            start=True, stop=True)
                    nc.scalar.activation(pt[:, k2:k2 + 2, :], sp2[:], AF.Exp)
                    if k2 >= 4 * qb:
                        j0 = k2 - 4 * qb
                        nc.vector.tensor_mul(pt[:, k2:k2 + 2, :],
                                             pt[:, k2:k2 + 2, :],
                                             mask_sb[:, j0:j0 + 2, :])
                    # incremental denominator: release pt pairs promptly
                    if k2 == 0:
                        nc.vector.tensor_add(acc[:], pt[:, 0, :], pt[:, 1, :])
                    elif k2 < nk - 2:
                        nc.vector.tensor_add(acc[:], acc[:], pt[:, k2, :])
                        nc.vector.tensor_add(acc[:], acc[:], pt[:, k2 + 1, :])
                    else:
                        nc.vector.tensor_add(acc[:], acc[:], pt[:, k2, :])
                        nc.vector.tensor_add(accb[:], acc[:],
                                             pt[:, k2 + 1, :])
                    if k2 >= 2:
                        av_pair(k2 - 2)
                av_pair(nk - 2)
                r = 4 * b + qb
                dnp = aps.tile([1, QR], F32, tag="dn", bufs=1)
                nc.tensor.matmul(dnp[:], ones_bf[:], accb[:],
                                 start=True, stop=True)
                nc.scalar.activation(dn_all[hd_t][b][32 * qb:32 * qb + 1, :],
                                     dnp[:], AF.Copy)
                nc.vector.tensor_scalar_add(av_keep[hd_t][:, r, :], avp[:],
                                            0.0)

        def norm_prep(hd_t, np_):
            rrow = np_.tile([1, 8 * QR], BF16, tag=f"rrow{hd_t}",
                            name=f"rrow{hd_t}", bufs=1)
            for b in range(B):
                rec = np_.tile([P, QR], F32, tag="rec", bufs=1)
                nc.vector.reciprocal(rec[:], dn_all[hd_t][b][:])
                recbf = np_.tile([P, QR], BF16, tag="recbf", bufs=1)
                nc.scalar.activation(recbf[:], rec[:], AF.Copy)
                for qb in range(4):
                    r = 4 * b + qb
                    nc.scalar.activation(rrow[0:1, QR * r:QR * (r + 1)],
                                         recbf[32 * qb:32 * qb + 1, :],
                                         AF.Copy)
            return rrow

        def norm_fire(hd_t, np_, rrow):
            with tc.tile_pool(name=f"nps{hd_t}", bufs=2, space="PSUM") as nps:
                for r in range(8):
                    bp = nps.tile([P, QR], F32, tag="brec", name=f"brc{hd_t}{r}")
                    nc.tensor.matmul(bp[:], ones_row_bf[:],
                                     rrow[0:1, QR * r:QR * (r + 1)],
                                     start=True, stop=True)
                    brec = np_.tile([P, QR], BF16, tag="brecs", bufs=1)
                    nc.scalar.activation(brec[:], bp[:], AF.Copy)
                    st = np_.tile([P, QR], BF16, tag="avst", bufs=1)
                    nc.vector.tensor_mul(st[:], av_keep[hd_t][:, r, :], brec[:])
                    nc.vector.tensor_scalar_add(st[:], st[:],
                                                bv_t[:, hd_t:hd_t + 1])
                    nc.sync.dma_start(a2a_in[hd_t][r], st[:])
            nc.gpsimd.collective_compute(
                "AllToAll", OP.bypass, replica_groups=[list(range(8))],
                ins=[a2a_in[hd_t][:].opt()], outs=[a2a_out[hd_t][:].opt()])
            nc.sync.dma_start(rx[:, 8 * hd_t:8 * (hd_t + 1), :],
                              a2a_out[hd_t].rearrange("j p q -> p j q"))

        # ---- schedule ----
        phase_a_apply_qkv(0)
        phase_a_apply_qkv(1)
        cm_l1.__exit__(None, None, None)
        cm_attn = tc.tile_pool(name="attn", bufs=1)
        atp = cm_attn.__enter__()
        mask_sb = atp.tile([P, 4, QR], BF16, tag="mask")
        nc.sync.dma_start(mask_sb[:], mask_h[:])
        cm_aps = tc.tile_pool(name="attnps", bufs=2, space="PSUM")
        aps = cm_aps.__enter__()
        attn_hb(0, 0, atp, aps)
        attn_hb(0, 1, atp, aps)
        cm_norm = tc.tile_pool(name="norm", bufs=1)
        np_ = cm_norm.__enter__()
        rrow0 = norm_prep(0, np_)
        attn_hb(1, 0, atp, aps)
        norm_fire(0, np_, rrow0)
        attn_hb(1, 1, atp, aps)
        rrow1 = norm_prep(1, np_)
        norm_fire(1, np_, rrow1)

        if "q0" in dbg:
            nc.sync.dma_start(dbg["q0"][:], q_sb[0][:])
        if "k0" in dbg:
            nc.sync.dma_start(dbg["k0"][:], k_sb[0][:])
        if "v0" in dbg:
            nc.sync.dma_start(dbg["v0"][:], vT_sb[0][:])
        for hh in range(2):
            if f"av{hh}" in dbg:
                nc.sync.dma_start(dbg[f"av{hh}"][:], av_keep[hh][:])

        cm_norm.__exit__(None, None, None)
        cm_aps.__exit__(None, None, None)
        cm_attn.__exit__(None, None, None)
        cm_av.__exit__(None, None, None)
        cm_qkv.__exit__(None, None, None)

        # ---- phase C: WO + residual, LN2, FFN ----
        cm_c = tc.tile_pool(name="cres", bufs=1)
        cp = cm_c.__enter__()
        if "rx" in dbg:
            nc.sync.dma_start(dbg["rx"][:], rx[:])
        h_bf = cp.tile([P, HT, QR], BF16, tag="hbf")
        woA = cp.tile([P, HT, QR], F32, tag="woA")
        with tc.tile_pool(name="wo", bufs=2) as wop, \
             tc.tile_pool(name="wops", bufs=2, space="PSUM") as wops:
            for dM in range(HT):
                wot = wop.tile([P, 8, P], BF16, tag="wotA")
                nc.sync.dma_start(wot[:], wo_h[dM][:, 0:8, :])
                ps = wops.tile([P, QR], F32, tag="wops")
                for kt in range(8):
                    nc.tensor.matmul(ps[:], wot[:, kt, :], rx[:, kt, :],
                                     start=(kt == 0), stop=(kt == 7))
                nc.scalar.activation(woA[:, dM, :], ps[:], AF.Copy)
            for dM in range(HT):
                wot = wop.tile([P, 8, P], BF16, tag="wotB")
                nc.sync.dma_start(wot[:], wo_h[dM][:, 8:16, :])
                xq_t = wop.tile([P, QR], F32, tag="xqt")
                nc.sync.dma_start(xq_t[:], xq32[:, dM, :])
                ps = wops.tile([P, QR], F32, tag="wops")
                for kt in range(8):
                    nc.tensor.matmul(ps[:], wot[:, kt, :], rx[:, 8 + kt, :],
                                     start=(kt == 0), stop=(kt == 7))
                tsum = wop.tile([P, QR], F32, tag="tsum")
                nc.vector.tensor_add(tsum[:], ps[:], woA[:, dM, :])
                nc.vector.scalar_tensor_tensor(
                    h_bf[:, dM, :], tsum[:], bwo_t[:, dM:dM + 1],
                    xq_t[:], op0=OP.add, op1=OP.add)
        if "h" in dbg:
            nc.sync.dma_start(dbg["h"][:], h_bf[:])

        g_sb = cp.tile([P, HT, QR], BF16, tag="g")
        with tc.tile_pool(name="ln2", bufs=1) as l2p:
            rowm2, rowr2 = _ln_stats_rows(nc, tc, l2p, ones_bf,
                                          lambda i: h_bf[:, i, :], QR, "l2")
            bmean2, brstd2 = _ln_bcast(nc, tc, l2p, ones_row_bf, rowm2, rowr2,
                                       QR, "l2")
            for i in range(HT):
                t1 = l2p.tile([P, QR], BF16, tag="t1", bufs=2)
                nc.vector.tensor_sub(t1[:], h_bf[:, i, :], bmean2[:])
                t2 = l2p.tile([P, QR], BF16, tag="t2", bufs=2)
                nc.vector.tensor_mul(t2[:], t1[:], brstd2[:])
                nc.scalar.activation(g_sb[:, i, :], t2[:], AF.Identity,
                                     bias=ln2b_t[:, i:i + 1],
                                     scale=ln2w_t[:, i:i + 1])
        if "g" in dbg:
            nc.sync.dma_start(dbg["g"][:], g_sb[:])

        f_sb = cp.tile([P, FT, QR], BF16, tag="f")
        with tc.tile_pool(name="ffn1", bufs=3) as f1p, \
             tc.tile_pool(name="f1ps", bufs=2, space="PSUM") as f1ps:
            for fM in range(FT):
                w1t = f1p.tile([P, HT, P], BF16, tag="w1t")
                nc.sync.dma_start(w1t[:], w1_h[fM])
                ps = f1ps.tile([P, QR], F32, tag="f1ps")
                for ht in range(HT):
                    nc.tensor.matmul(ps[:], w1t[:, ht, :], g_sb[:, ht, :],
                                     start=(ht == 0), stop=(ht == HT - 1))
                nc.scalar.activation(f_sb[:, fM, :], ps[:], AF.Gelu,
                                     bias=b1_t[:, fM:fM + 1])

        with tc.tile_pool(name="ffn2", bufs=2) as f2p, \
             tc.tile_pool(name="f2ps", bufs=2, space="PSUM") as f2ps:
            for dM in range(HT):
                ps = f2ps.tile([P, QR], F32, tag="f2ps")
                for q4 in range(4):
                    w2t = f2p.tile([P, HT, P], BF16, tag="w2t")
                    nc.sync.dma_start(w2t[:], w2_h[dM][:, HT * q4:HT * (q4 + 1), :])
                    for ft in range(HT):
                        kk = HT * q4 + ft
                        nc.tensor.matmul(ps[:], w2t[:, ft, :], f_sb[:, kk, :],
                                         start=(kk == 0), stop=(kk == FT - 1))
                ost = f2p.tile([P, QR], F32, tag="ost")
                nc.vector.scalar_tensor_tensor(
                    ost[:], ps[:], b2_t[:, dM:dM + 1], h_bf[:, dM, :],
                    op0=OP.add, op1=OP.add)
                nc.sync.dma_start(outT[P * dM:P * (dM + 1), :], ost[:])
        cm_c.__exit__(None, None, None)
        cm_rx.__exit__(None, None, None)
        cm_const.__exit__(None, None, None)

    return nc


# ---------------------------------------------------------------------------
# Host side
# ---------------------------------------------------------------------------

_CACHE = {}


def _get_nc(debug_outputs=()):
    key = tuple(sorted(debug_outputs))
    if key not in _CACHE:
        _CACHE[key] = build_nc(debug_outputs)
    return _CACHE[key]


def tile_kxm(wT):
    # [K, M] -> [mM, p(K), kt, m2] so each [128, kt*128] lhsT load is
    # contiguous per partition.
    K_, M_ = wT.shape
    return np.ascontiguousarray(
        wT.reshape(K_ // P, P, M_ // P, P).transpose(2, 1, 0, 3))


def pcol(v):
    # [n*P] -> [P, n] bias tile (column m holds elements m*P..m*P+P-1)
    return np.ascontiguousarray(
        np.asarray(v, np.float32).reshape(-1, P).T)


def make_in_maps(inputs):
    x = np.asarray(inputs["x"], np.float32)
    scale = np.float32(1.0 / np.sqrt(HD))
    wqkv = np.asarray(inputs["wqkv_w"], np.float32)
    wqkv_b = np.asarray(inputs["wqkv_b"], np.float32)

    xT_t = np.ascontiguousarray(
        x.transpose(0, 2, 1).reshape(B, HT, P, S).transpose(0, 2, 1, 3)
    ).astype(NPBF)

    mask = np.zeros((P, 4, QR), np.float32)
    r_ = np.arange(P)[:, None]
    c_ = np.arange(QR)[None, :]
    for j in range(4):
        mask[:, j, :] = (P * j + r_ <= c_).astype(np.float32)

    shared = {
        "xT_t": xT_t,
        "mask_h": mask.astype(NPBF),
        "ones_d": np.ones((P, 1), NPBF),
        "ones_r_d": np.ones((1, P), NPBF),
        "wo_h": tile_kxm(np.asarray(inputs["wo_w"], np.float32).T).astype(NPBF),
        "w1_h": tile_kxm(np.asarray(inputs["w1"], np.float32).T).astype(NPBF),
        "w2_h": tile_kxm(np.asarray(inputs["w2"], np.float32).T).astype(NPBF),
        "bwo_d": pcol(inputs["wo_b"]),
        "b1_d": pcol(inputs["b1"]),
        "b2_d": pcol(inputs["b2"]),
        "ln1w_d": pcol(inputs["ln1_w"]),
        "ln1b_d": pcol(inputs["ln1_b"]),
        "ln2w_d": pcol(inputs["ln2_w"]),
        "ln2b_d": pcol(inputs["ln2_b"]),
    }
    in_maps = []
    for core in range(8):
        rows = np.r_[core * P:(core + 1) * P,
                     (core + 8) * P:(core + 9) * P]
        wq_s = wqkv[:H][rows] * scale
        wk_s = wqkv[H:2 * H][rows]
        wv_s = wqkv[2 * H:][rows]
        b_out, c_out = divmod(core, 4)
        xq = x[b_out, QR * c_out:QR * (c_out + 1), :].T  # [H, QR]
        xq_t = np.ascontiguousarray(xq.reshape(HT, P, QR).transpose(1, 0, 2))
        in_maps.append(dict(
            shared,
            xq32=xq_t,
            xstat_d=xq_t.astype(NPBF),
            wq_h=np.ascontiguousarray(
                tile_kxm(wq_s.T).transpose(1, 0, 2, 3)).astype(NPBF),
            wk_h=np.ascontiguousarray(
                tile_kxm(wk_s.T).transpose(1, 0, 2, 3)).astype(NPBF),
            wvT_h=np.ascontiguousarray(
                wv_s.T.reshape(HT, P, 2 * P).transpose(1, 0, 2)).astype(NPBF),
            bq_d=np.ascontiguousarray(
                (wqkv_b[:H][rows] * scale).reshape(2, P).T),
            bk_d=np.ascontiguousarray(wqkv_b[H:2 * H][rows].reshape(2, P).T),
            bv_d=np.ascontiguousarray(wqkv_b[2 * H:][rows].reshape(2, P).T),
        ))
    return in_maps


def run_cores(inputs, debug_outputs=(), **run_kw):
    nc = _get_nc(debug_outputs)
    in_maps = make_in_maps(inputs)
    return nc, run_bass_kernel_spmd(nc, in_maps, core_ids=list(range(8)),
                                    **run_kw)


def kernel(**inputs):
    _, res = run_cores(inputs)
    out = np.empty((B, S, H), np.float32)
    for core in range(8):
        b, c = divmod(core, 4)
        out[b, QR * c:QR * (c + 1), :] = res.results[core]["outT"].T
    return out
